# revision 2
# baseline (speedup 1.0000x reference)
"""Llama3 attention prefill kernel for 8 Trainium2 NeuronCores.

Sharding: tensor-parallel over heads. Core c owns Q heads 4c..4c+3 and KV
head c (GQA group), plus the matching wqkv columns / wo rows. Each core
computes a partial output y_c = attn_c @ wo_c; the host sums the partials.

Schedule (single TileContext, one long PE stream with software pipelining):
  0. PE warm-up dummies at t=0 ride out the 0.65->2.4 GHz DVFS ramp while
     the first w/x DMA chunks land.
  1. Supertiles st=0..2: qkvT = w^T x in transposed layout (q/k) plus v in
     natural [pos, d] layout via lhsT=x; eager per-column PSUM eviction +
     RoPE on DVE.  g=0's flash-attention steps interleave into st=2.
  2. Overlap window: st=3's qkv runs as sequential per-column streams
     (2 PSUM banks) while g=1 and g=2 attention steps pump between chunks.
  3. g=3 attention + all y^T = wo^T out^T tiles interleaved; the kernel
     tail is a single y tile evict+DMA.
  Attention: S^T = K_j^T Q_g per (k-block, 512-wide q group) -> exp gives
  P^T directly, causal diagonal via multiplicative mask, row sums on
  gpsimd, normalization fused into the outT eviction multiply.
"""

import os
import sys

for _p in ("/opt/trn_rl_repo", "/root/.axon_site/_ro/trn_rl_repo"):
    if os.path.isdir(_p) and _p not in sys.path:
        sys.path.insert(0, _p)

import numpy as np

S = 2048
H = 4096
HD = 128
NQ = 4            # q heads per core
MQKV = 768        # per-core qkv columns: 512 q + 128 k + 128 v
N_CORES = 8
KC = H // 128     # 32 contraction chunks for qkv
KT = S // 128     # 16 pos tiles
NG = S // 512     # 4 q groups of 512 positions
SCALE = 1.0 / float(np.sqrt(HD))

_CACHE = {}
LAST_RESULTS = None


def _build():
    import concourse.tile as tile
    from concourse import bacc, bass_isa, mybir

    f32 = mybir.dt.float32
    f16 = mybir.dt.float16
    Exp = mybir.ActivationFunctionType.Exp

    nc = bacc.Bacc("TRN2", target_bir_lowering=False, debug=False)

    xT_ap = nc.dram_tensor("xT", [H, S], f16, kind="ExternalInput").ap()
    w_ap = nc.dram_tensor("w", [H, MQKV], f16, kind="ExternalInput").ap()
    wo_ap = nc.dram_tensor("wo", [NQ * HD, H], f16, kind="ExternalInput").ap()
    cs_ap = nc.dram_tensor("cs2", [128, S], f16, kind="ExternalInput").ap()
    sn_ap = nc.dram_tensor("sn2", [128, S], f16, kind="ExternalInput").ap()
    dm_ap = nc.dram_tensor("dmask", [128, 128], f16, kind="ExternalInput").ap()
    yT_ap = nc.dram_tensor("yT", [H, S], f16, kind="ExternalOutput").ap()

    # DRAM views for batched DMA: [p, chunk, col]
    xT_v = xT_ap.rearrange("(a p) s -> p a s", p=128)      # [128, 32, 2048]
    w_v = w_ap.rearrange("(a p) c -> p a c", p=128)        # [128, 32, 768]
    wo_v = wo_ap.rearrange("(a p) c -> p a c", p=128)      # [128, 4, 4096]
    yT_v = yT_ap.rearrange("(a p) s -> p a s", p=128)      # [128, 32, 2048]

    with tile.TileContext(nc) as tc:
        from contextlib import ExitStack

        with ExitStack() as ctx:
            const = ctx.enter_context(tc.tile_pool(name="const", bufs=1))
            dmask = const.tile([128, 128], f16)
            # warm-up dummies read dmask before its DMA lands (WAR is
            # sequenced by the framework); contents are irrelevant.
            nc.vector.memset(dmask[:], 0.0)

            # resident tensors
            res = ctx.enter_context(tc.tile_pool(name="res", bufs=1))
            qT_sb = res.tile([128, NQ, S], f16, name="qT_sb")
            kT_sb = res.tile([128, S], f16, name="kT_sb")
            v_sb = res.tile([128, KT, 128], f16, name="v_sb")
            outT_sb = res.tile([128, NQ, S], f16, name="outT_sb")

            # wo resident from the start; DMA'd during phase 1.
            # Right-side stack order: pools that die mid-program (w) sit on
            # top of the program-lifetime attention pools (pool frees are
            # LIFO per side).
            wo_pool = ctx.enter_context(
                tc.tile_pool(name="wo_pool", bufs=1, side="right"))
            wo_sb = wo_pool.tile([128, NQ, H], f16, name="wo_sb")

            # attention SBUF pools (live from st2 through the end)
            pp = ctx.enter_context(
                tc.tile_pool(name="pp", bufs=3, side="right"))
            rr = ctx.enter_context(
                tc.tile_pool(name="rr", bufs=2, side="right"))
            lp = ctx.enter_context(
                tc.tile_pool(name="lp", bufs=1, side="right"))

            w_pool_cm = tc.tile_pool(name="w_pool", bufs=1, side="right")
            w_pool = w_pool_cm.__enter__()
            w_sb = w_pool.tile([128, KC, MQKV], f16, name="w_sb")

            xb_pool_cm = tc.tile_pool(name="xb", bufs=2)
            xb_pool = xb_pool_cm.__enter__()

            # rope streaming pools (cs/sn stream per supertile: 2-deep)
            csn_cm = tc.tile_pool(name="csn", bufs=2)
            csn = csn_cm.__enter__()
            ep_cm = tc.tile_pool(name="ep", bufs=2)
            ep = ep_cm.__enter__()
            rp_cm = tc.tile_pool(name="rp", bufs=2)
            rp = rp_cm.__enter__()

            # PSUM: scores(2) + v(1) + qkv(5) = 8 during st0-2
            ps_s = ctx.enter_context(
                tc.tile_pool(name="ps_s", bufs=2, space="PSUM"))
            vps_cm = tc.tile_pool(name="ps_v", bufs=1, space="PSUM")
            vps = vps_cm.__enter__()
            ps1_cm = tc.tile_pool(name="ps_qkv", bufs=5, space="PSUM")
            ps1 = ps1_cm.__enter__()

            # PE warm-up: the tensor engine ramps 0.65->1.2->2.4 GHz over
            # ~3us of continuous execution.  Dummy matmuls that depend only
            # on the memset ride out the ramp while the first w/x DMA
            # chunks are in flight; results are never read.
            wps = vps.tile([128, 4, 128], f32, tag="vt", name="warm_ps")
            for wi in range(36):
                nc.tensor.matmul(wps[:, 0, :], lhsT=dmask[:], rhs=dmask[:],
                                 start=True, stop=True)

            # ---- DMA schedule (single shared engine; order = priority) ----
            xbufs = [xb_pool.tile([128, KC, 512], f16, tag="xb",
                                  name=f"xb{st}") for st in range(NG)]
            _edges = [0, 2, 4, 6, 8, 12, 16, 20, 24, 28, 32]
            for _a, _b in zip(_edges[:-1], _edges[1:]):
                ksl = slice(_a, _b)
                nc.sync.dma_start(out=w_sb[:, ksl, :], in_=w_v[:, ksl, :])
                nc.sync.dma_start(out=xbufs[0][:, ksl, :],
                                  in_=xT_v[:, ksl, 0:512])
            for k8 in range(0, KC, 8):
                nc.sync.dma_start(out=xbufs[1][:, k8:k8 + 8, :],
                                  in_=xT_v[:, k8:k8 + 8, 512:1024])
            cs_ts, sn_ts = [], []
            for st in range(NG):
                cs_ts.append(csn.tile([128, 512], f16, tag="cs",
                                      name=f"cs{st}"))
                sn_ts.append(csn.tile([128, 512], f16, tag="sn",
                                      name=f"sn{st}"))
            stsl = [slice(st * 512, (st + 1) * 512) for st in range(NG)]
            # cs/sn for st0/st1 land before their rope; st2/st3 slices reuse
            # the buffers, so their DMAs (which wait on st0/st1 rope) are
            # queued last to keep the in-order DMA queue from head-blocking
            for st in (0, 1):
                nc.sync.dma_start(out=cs_ts[st][:], in_=cs_ap[:, stsl[st]])
                nc.sync.dma_start(out=sn_ts[st][:], in_=sn_ap[:, stsl[st]])
            nc.sync.dma_start(out=dmask[:], in_=dm_ap[:, :])
            nc.sync.dma_start(out=xbufs[2][:], in_=xT_v[:, :, 1024:1536])
            nc.sync.dma_start(out=xbufs[3][:], in_=xT_v[:, :, 1536:2048])
            nc.sync.dma_start(out=wo_sb[:], in_=wo_v[:, :, :])
            for st in (2, 3):
                nc.sync.dma_start(out=cs_ts[st][:], in_=cs_ap[:, stsl[st]])
                nc.sync.dma_start(out=sn_ts[st][:], in_=sn_ap[:, stsl[st]])

            # ---- helpers ----
            def evict_c6(st, c6, qps, sl):
                # rope: partner half via two half-partition copies
                E = ep.tile([128, 512], f16, tag="E")
                nc.scalar.copy(out=E[:], in_=qps[:])
                Esw = ep.tile([128, 512], f16, tag="Esw")
                nc.vector.tensor_copy(out=Esw[0:64, :], in_=E[64:128, :])
                nc.vector.tensor_copy(out=Esw[64:128, :], in_=E[0:64, :])
                t1 = rp.tile([128, 512], f16, tag="t1")
                nc.vector.tensor_mul(t1[:], E[:], cs_ts[st][:])
                t2 = rp.tile([128, 512], f16, tag="t2")
                nc.vector.tensor_mul(t2[:], Esw[:], sn_ts[st][:])
                dst = (qT_sb[:, c6, sl] if c6 < 4 else kT_sb[:, sl])
                nc.vector.tensor_add(dst, t1[:], t2[:])

            def attn_step(g, h, j, ops, R, sp_pool):
                jmax = 4 * g + 3
                ing = (j // 4 == g)
                c0 = (j - 4 * g) * 128 if ing else 0
                sps = sp_pool.tile([128, 512], f32, tag="sps")
                nc.tensor.matmul(
                    sps[:, c0:],
                    lhsT=kT_sb[:, j * 128:(j + 1) * 128],
                    rhs=qT_sb[:, h, g * 512 + c0:(g + 1) * 512],
                    start=True, stop=True)
                P = pp.tile([128, 512], f16, tag="P")
                nc.scalar.activation(P[:, c0:], sps[:, c0:], Exp, scale=SCALE)
                if ing:
                    nc.vector.tensor_mul(
                        P[:, c0:c0 + 128], P[:, c0:c0 + 128], dmask[:])
                if j == 0:
                    nc.vector.tensor_copy(out=R[:], in_=P[:])
                else:
                    nc.vector.tensor_add(R[:, c0:], R[:, c0:], P[:, c0:])
                nc.tensor.matmul(
                    ops[:, c0:], lhsT=v_sb[:, j, :], rhs=P[:, c0:],
                    start=(j == 0), stop=(j == jmax))

            def attn_tail(g, h, ops, R):
                # softmax denominators on Pool/DVE only; normalization is
                # fused into the outT eviction multiply
                gsl = slice(g * 512, (g + 1) * 512)
                l_bc = lp.tile([128, 512], f32, tag="lbc")
                nc.gpsimd.partition_all_reduce(
                    l_bc[:], R[:], channels=128,
                    reduce_op=bass_isa.ReduceOp.add)
                rbc = lp.tile([128, 512], f16, tag="rbc_sb")
                with nc.allow_low_precision(reason="1/l fits f16"):
                    nc.vector.reciprocal(rbc[:], l_bc[:])
                nc.vector.tensor_mul(outT_sb[:, h, gsl], ops[:], rbc[:])

            def attn_stream(groups, ops_pool, sp_pool):
                # generator: one yield per emitted (S, exp, PV) j-step
                for g in groups:
                    for h in range(NQ):
                        ops = ops_pool.tile([128, 512], f32, tag="ops")
                        R = rr.tile([128, 512], f16, tag="R")
                        for j in range(4 * g + 4):
                            attn_step(g, h, j, ops, R, sp_pool)
                            yield
                        attn_tail(g, h, ops, R)

            def pump(stream, n):
                for _ in range(n):
                    if next(stream, "done") == "done":
                        return

            # ---- supertiles 0..2: qkv + rope ----
            for st in range(3):
                xb = xbufs[st]
                sl = slice(st * 512, (st + 1) * 512)
                qpss = [ps1.tile([128, 512], f32, tag="qkv",
                                 name=f"qps{st}_{c6}") for c6 in range(5)]
                vt = vps.tile([128, 4, 128], f32, tag="vt", name=f"vps{st}")
                for kq in range(7):
                    for c6 in range(5):
                        for kc in range(kq * 4, kq * 4 + 4):
                            nc.tensor.matmul(
                                qpss[c6][:],
                                lhsT=w_sb[:, kc, c6 * 128:(c6 + 1) * 128],
                                rhs=xb[:, kc, :],
                                start=(kc == 0), stop=False)
                    for kc in range(kq * 4, kq * 4 + 4):
                        for q in range(4):
                            # start only on the first matmul into the bank:
                            # start=True zeroes the whole 2KB region, which
                            # pre-zeroes all four q-group slices
                            nc.tensor.matmul(
                                vt[:, q, :],
                                lhsT=xb[:, kc, q * 128:(q + 1) * 128],
                                rhs=w_sb[:, kc, 640:768],
                                start=(kc == 0 and q == 0), stop=False)
                for c6 in range(5):
                    for kc in range(28, 32):
                        nc.tensor.matmul(
                            qpss[c6][:],
                            lhsT=w_sb[:, kc, c6 * 128:(c6 + 1) * 128],
                            rhs=xb[:, kc, :],
                            start=False, stop=(kc == KC - 1))
                    evict_c6(st, c6, qpss[c6], sl)
                for kc in range(28, 32):
                    for q in range(4):
                        nc.tensor.matmul(
                            vt[:, q, :],
                            lhsT=xb[:, kc, q * 128:(q + 1) * 128],
                            rhs=w_sb[:, kc, 640:768],
                            start=False, stop=(kc == KC - 1))
                nc.scalar.copy(out=v_sb[:, st * 4:(st + 1) * 4, :],
                               in_=vt[:])

            # ---- overlap window: st3 qkv as sequential per-column streams,
            # g0+g1+g2 attention pumped between chunks ----
            ps1_cm.__exit__(None, None, None)
            ops2_cm = tc.tile_pool(name="ps_o2", bufs=2, space="PSUM")
            ops2 = ops2_cm.__enter__()
            ps1b_cm = tc.tile_pool(name="ps_qkv3", bufs=3, space="PSUM")
            ps1b = ps1b_cm.__enter__()

            g12_stream = attn_stream([0, 1, 2], ops2, ps_s)
            st, xb, sl = 3, xbufs[3], slice(1536, 2048)
            vt = vps.tile([128, 4, 128], f32, tag="vt", name="vps3")
            for c6 in range(5):
                qps = ps1b.tile([128, 512], f32, tag="qkv3")
                for k4 in range(0, KC, 4):
                    for kc in range(k4, k4 + 4):
                        nc.tensor.matmul(
                            qps[:],
                            lhsT=w_sb[:, kc, c6 * 128:(c6 + 1) * 128],
                            rhs=xb[:, kc, :],
                            start=(kc == 0), stop=(kc == KC - 1))
                    pump(g12_stream, 2)
                evict_c6(st, c6, qps, sl)
            for k4 in range(0, KC, 4):
                for kc in range(k4, k4 + 4):
                    for q in range(4):
                        nc.tensor.matmul(
                            vt[:, q, :],
                            lhsT=xb[:, kc, q * 128:(q + 1) * 128],
                            rhs=w_sb[:, kc, 640:768],
                            start=(kc == 0 and q == 0),
                            stop=(kc == KC - 1))
                pump(g12_stream, 2)
            nc.scalar.copy(out=v_sb[:, 12:16, :], in_=vt[:])
            pump(g12_stream, 999)  # drain leftover steps

            # ---- free phase-1 pools; g3 attention + all y tiles ----
            ps1b_cm.__exit__(None, None, None)
            rp_cm.__exit__(None, None, None)
            ep_cm.__exit__(None, None, None)
            csn_cm.__exit__(None, None, None)
            xb_pool_cm.__exit__(None, None, None)
            w_pool_cm.__exit__(None, None, None)

            yp_cm = tc.tile_pool(name="yp", bufs=6)
            yp = yp_cm.__enter__()
            ps_y_cm = tc.tile_pool(name="ps_y", bufs=3, space="PSUM")
            ps_y = ps_y_cm.__enter__()

            ycnt = 0

            def y_tile(gy, ym):
                nonlocal ycnt
                gysl = slice(gy * 512, (gy + 1) * 512)
                yps = ps_y.tile([128, 512], f32, tag="yps")
                for kc in range(NQ):
                    nc.tensor.matmul(
                        yps[:],
                        lhsT=wo_sb[:, kc, ym * 128:(ym + 1) * 128],
                        rhs=outT_sb[:, kc, gysl],
                        start=(kc == 0), stop=(kc == NQ - 1))
                ysm = yp.tile([128, 512], f16, tag="ysm")
                if ycnt % 2 == 0:
                    nc.scalar.copy(out=ysm[:], in_=yps[:])
                else:
                    nc.vector.tensor_copy(out=ysm[:], in_=yps[:])
                nc.sync.dma_start(out=yT_v[:, ym:ym + 1, gysl], in_=ysm[:])
                ycnt += 1

            yq = [(g, ym) for g in range(3) for ym in range(32)]
            yi = 0

            def pump_y(n):
                nonlocal yi
                for _ in range(n):
                    if yi < len(yq):
                        y_tile(*yq[yi])
                        yi += 1

            for h in range(NQ):
                ops = ops2.tile([128, 512], f32, tag="ops")
                R = rr.tile([128, 512], f16, tag="R")
                for j in range(16):
                    attn_step(3, h, j, ops, R, ps_s)
                    pump_y(1)
                attn_tail(3, h, ops, R)
                pump_y(2)
            yq.extend((3, ym) for ym in range(32))
            pump_y(len(yq))

            ps_y_cm.__exit__(None, None, None)
            yp_cm.__exit__(None, None, None)
            ops2_cm.__exit__(None, None, None)
            vps_cm.__exit__(None, None, None)

    nc.compile()
    return nc


def _get_nc():
    if "nc" not in _CACHE:
        _CACHE["nc"] = _build()
    return _CACHE["nc"]


def _prep_inputs(x, rope_cache, wqkv, wo):
    x2 = np.asarray(x, np.float32).reshape(S, H)
    xT = np.ascontiguousarray(x2.T.astype(np.float16))          # [H, S]

    rc = np.asarray(rope_cache, np.float32)                      # [S, 64, 2]
    c = rc[:, :, 0].T.astype(np.float16)                         # [64, S]
    s = rc[:, :, 1].T.astype(np.float16)
    cs2 = np.ascontiguousarray(np.vstack([c, c]))                # [128, S]
    sn2 = np.ascontiguousarray(np.vstack([-s, s]))

    # transposed causal mask (multiplicative): keep k <= q
    kk = np.arange(128)
    dmask = (kk[:, None] <= kk[None, :]).astype(np.float16)

    # rope pair permutation within each 128-wide head: evens then odds
    perm = np.concatenate([np.arange(0, 128, 2), np.arange(1, 128, 2)])

    wq = np.asarray(wqkv, np.float32)
    wo_f = np.asarray(wo, np.float32)

    in_maps = []
    for cid in range(N_CORES):
        cols = []
        for hh in range(NQ):
            qh = wq[:, cid * 512 + hh * 128: cid * 512 + (hh + 1) * 128]
            cols.append(qh[:, perm])
        kh = wq[:, H + cid * 128: H + (cid + 1) * 128]
        cols.append(kh[:, perm])
        vh = wq[:, H + 1024 + cid * 128: H + 1024 + (cid + 1) * 128]
        cols.append(vh)
        wcat = np.concatenate(cols, axis=1).astype(np.float16)
        in_maps.append({
            "xT": xT,
            "w": np.ascontiguousarray(wcat),
            "wo": np.ascontiguousarray(
                wo_f[cid * 512:(cid + 1) * 512, :].astype(np.float16)),
            "cs2": cs2,
            "sn2": sn2,
            "dmask": dmask,
        })
    return in_maps


def kernel(x, last_pos, mask, rope_cache, wqkv, wo):
    global LAST_RESULTS
    from concourse.bass_utils import run_bass_kernel_spmd

    nc = _get_nc()
    in_maps = _prep_inputs(x, rope_cache, wqkv, wo)

    res = run_bass_kernel_spmd(nc, in_maps, list(range(N_CORES)))
    LAST_RESULTS = res
    if res.exec_time_ns is not None:
        print(f"HW exec time: {res.exec_time_ns} ns")
    yT = res.results[0]["yT"].astype(np.float64)
    for c in range(1, N_CORES):
        yT = yT + res.results[c]["yT"]
    return np.ascontiguousarray(yT.T).reshape(1, S, H).astype(np.float32)


# revision 3
# speedup vs baseline: 1.0022x; 1.0022x over previous
"""Llama3 attention prefill kernel for 8 Trainium2 NeuronCores.

Sharding: tensor-parallel over heads. Core c owns Q heads 4c..4c+3 and KV
head c (GQA group), plus the matching wqkv columns / wo rows. Each core
computes a partial output y_c = attn_c @ wo_c; the host sums the partials.

Schedule (single TileContext, one long PE stream with software pipelining):
  0. PE warm-up dummies at t=0 ride out the 0.65->2.4 GHz DVFS ramp while
     the first w/x DMA chunks land.
  1. Supertiles st=0..2: qkvT = w^T x in transposed layout (q/k) plus v in
     natural [pos, d] layout via lhsT=x; eager per-column PSUM eviction +
     RoPE on DVE.  g=0's flash-attention steps interleave into st=2.
  2. Overlap window: st=3's qkv runs as sequential per-column streams
     (2 PSUM banks) while g=1 and g=2 attention steps pump between chunks.
  3. g=3 attention + all y^T = wo^T out^T tiles interleaved; the kernel
     tail is a single y tile evict+DMA.
  Attention: S^T = K_j^T Q_g per (k-block, 512-wide q group) -> exp gives
  P^T directly, causal diagonal via multiplicative mask, row sums on
  gpsimd, normalization fused into the outT eviction multiply.
"""

import os
import sys

for _p in ("/opt/trn_rl_repo", "/root/.axon_site/_ro/trn_rl_repo"):
    if os.path.isdir(_p) and _p not in sys.path:
        sys.path.insert(0, _p)

import numpy as np

S = 2048
H = 4096
HD = 128
NQ = 4            # q heads per core
MQKV = 768        # per-core qkv columns: 512 q + 128 k + 128 v
N_CORES = 8
KC = H // 128     # 32 contraction chunks for qkv
KT = S // 128     # 16 pos tiles
NG = S // 512     # 4 q groups of 512 positions
SCALE = 1.0 / float(np.sqrt(HD))

_CACHE = {}
LAST_RESULTS = None


def _build():
    import concourse.tile as tile
    from concourse import bacc, bass_isa, mybir

    f32 = mybir.dt.float32
    f16 = mybir.dt.float16
    Exp = mybir.ActivationFunctionType.Exp

    nc = bacc.Bacc("TRN2", target_bir_lowering=False, debug=False)

    xT_ap = nc.dram_tensor("xT", [H, S], f16, kind="ExternalInput").ap()
    w_ap = nc.dram_tensor("w", [H, MQKV], f16, kind="ExternalInput").ap()
    wo_ap = nc.dram_tensor("wo", [NQ * HD, H], f16, kind="ExternalInput").ap()
    cs_ap = nc.dram_tensor("cs2", [128, S], f16, kind="ExternalInput").ap()
    sn_ap = nc.dram_tensor("sn2", [128, S], f16, kind="ExternalInput").ap()
    dm_ap = nc.dram_tensor("dmask", [128, 128], f16, kind="ExternalInput").ap()
    yT_ap = nc.dram_tensor("yT", [H, S], f16, kind="ExternalOutput").ap()

    # DRAM views for batched DMA: [p, chunk, col]
    xT_v = xT_ap.rearrange("(a p) s -> p a s", p=128)      # [128, 32, 2048]
    w_v = w_ap.rearrange("(a p) c -> p a c", p=128)        # [128, 32, 768]
    wo_v = wo_ap.rearrange("(a p) c -> p a c", p=128)      # [128, 4, 4096]
    yT_v = yT_ap.rearrange("(a p) s -> p a s", p=128)      # [128, 32, 2048]

    with tile.TileContext(nc) as tc:
        from contextlib import ExitStack

        with ExitStack() as ctx:
            const = ctx.enter_context(tc.tile_pool(name="const", bufs=1))
            dmask = const.tile([128, 128], f16)
            # warm-up dummies read dmask before its DMA lands (WAR is
            # sequenced by the framework); contents are irrelevant.
            nc.gpsimd.memset(dmask[:], 0.0)

            # resident tensors
            res = ctx.enter_context(tc.tile_pool(name="res", bufs=1))
            qT_sb = res.tile([128, NQ, S], f16, name="qT_sb")
            kT_sb = res.tile([128, S], f16, name="kT_sb")
            v_sb = res.tile([128, KT, 128], f16, name="v_sb")
            outT_sb = res.tile([128, NQ, S], f16, name="outT_sb")

            # wo resident from the start; DMA'd during phase 1.
            # Right-side stack order: pools that die mid-program (w) sit on
            # top of the program-lifetime attention pools (pool frees are
            # LIFO per side).
            wo_pool = ctx.enter_context(
                tc.tile_pool(name="wo_pool", bufs=1, side="right"))
            wo_sb = wo_pool.tile([128, NQ, H], f16, name="wo_sb")

            # attention SBUF pools (live from st2 through the end)
            pp = ctx.enter_context(
                tc.tile_pool(name="pp", bufs=3, side="right"))
            rr = ctx.enter_context(
                tc.tile_pool(name="rr", bufs=2, side="right"))
            lp = ctx.enter_context(
                tc.tile_pool(name="lp", bufs=1, side="right"))

            w_pool_cm = tc.tile_pool(name="w_pool", bufs=1, side="right")
            w_pool = w_pool_cm.__enter__()
            w_sb = w_pool.tile([128, KC, MQKV], f16, name="w_sb")

            xb_pool_cm = tc.tile_pool(name="xb", bufs=2)
            xb_pool = xb_pool_cm.__enter__()

            # rope streaming pools (cs/sn stream per supertile: 2-deep)
            csn_cm = tc.tile_pool(name="csn", bufs=2)
            csn = csn_cm.__enter__()
            ep_cm = tc.tile_pool(name="ep", bufs=2)
            ep = ep_cm.__enter__()
            rp_cm = tc.tile_pool(name="rp", bufs=2)
            rp = rp_cm.__enter__()

            # PSUM: scores(2) + v(1) + qkv(5) = 8 during st0-2
            ps_s = ctx.enter_context(
                tc.tile_pool(name="ps_s", bufs=2, space="PSUM"))
            vps_cm = tc.tile_pool(name="ps_v", bufs=1, space="PSUM")
            vps = vps_cm.__enter__()
            ps1_cm = tc.tile_pool(name="ps_qkv", bufs=5, space="PSUM")
            ps1 = ps1_cm.__enter__()

            # PE warm-up: the tensor engine ramps 0.65->1.2->2.4 GHz over
            # ~3us of continuous execution.  Dummy matmuls that depend only
            # on the memset ride out the ramp while the first w/x DMA
            # chunks are in flight; results are never read.
            wps = vps.tile([128, 4, 128], f32, tag="vt", name="warm_ps")
            for wi in range(28):
                nc.tensor.matmul(wps[:, 0, :], lhsT=dmask[:], rhs=dmask[:],
                                 start=True, stop=True)

            # ---- DMA schedule (single shared engine; order = priority) ----
            xbufs = [xb_pool.tile([128, KC, 512], f16, tag="xb",
                                  name=f"xb{st}") for st in range(NG)]
            _edges = [0, 1, 2, 4, 6, 8, 12, 16, 20, 24, 28, 32]
            for _a, _b in zip(_edges[:-1], _edges[1:]):
                ksl = slice(_a, _b)
                nc.sync.dma_start(out=w_sb[:, ksl, :], in_=w_v[:, ksl, :])
                nc.sync.dma_start(out=xbufs[0][:, ksl, :],
                                  in_=xT_v[:, ksl, 0:512])
            for k8 in range(0, KC, 8):
                nc.sync.dma_start(out=xbufs[1][:, k8:k8 + 8, :],
                                  in_=xT_v[:, k8:k8 + 8, 512:1024])
            cs_ts, sn_ts = [], []
            for st in range(NG):
                cs_ts.append(csn.tile([128, 512], f16, tag="cs",
                                      name=f"cs{st}"))
                sn_ts.append(csn.tile([128, 512], f16, tag="sn",
                                      name=f"sn{st}"))
            stsl = [slice(st * 512, (st + 1) * 512) for st in range(NG)]
            # cs/sn for st0/st1 land before their rope; st2/st3 slices reuse
            # the buffers, so their DMAs (which wait on st0/st1 rope) are
            # queued last to keep the in-order DMA queue from head-blocking
            for st in (0, 1):
                nc.sync.dma_start(out=cs_ts[st][:], in_=cs_ap[:, stsl[st]])
                nc.sync.dma_start(out=sn_ts[st][:], in_=sn_ap[:, stsl[st]])
            nc.sync.dma_start(out=dmask[:], in_=dm_ap[:, :])
            nc.sync.dma_start(out=xbufs[2][:], in_=xT_v[:, :, 1024:1536])
            nc.sync.dma_start(out=xbufs[3][:], in_=xT_v[:, :, 1536:2048])
            nc.sync.dma_start(out=wo_sb[:], in_=wo_v[:, :, :])
            for st in (2, 3):
                nc.sync.dma_start(out=cs_ts[st][:], in_=cs_ap[:, stsl[st]])
                nc.sync.dma_start(out=sn_ts[st][:], in_=sn_ap[:, stsl[st]])

            # ---- helpers ----
            def evict_c6(st, c6, qps, sl):
                # rope: partner half via two half-partition copies
                E = ep.tile([128, 512], f16, tag="E")
                nc.scalar.copy(out=E[:], in_=qps[:])
                Esw = ep.tile([128, 512], f16, tag="Esw")
                nc.vector.tensor_copy(out=Esw[0:64, :], in_=E[64:128, :])
                nc.vector.tensor_copy(out=Esw[64:128, :], in_=E[0:64, :])
                t1 = rp.tile([128, 512], f16, tag="t1")
                nc.vector.tensor_mul(t1[:], E[:], cs_ts[st][:])
                t2 = rp.tile([128, 512], f16, tag="t2")
                nc.vector.tensor_mul(t2[:], Esw[:], sn_ts[st][:])
                dst = (qT_sb[:, c6, sl] if c6 < 4 else kT_sb[:, sl])
                nc.vector.tensor_add(dst, t1[:], t2[:])

            def attn_p(g, h, j, R, sp_pool):
                # P production: S^T matmul -> exp -> causal mask -> R add
                ing = (j // 4 == g)
                c0 = (j - 4 * g) * 128 if ing else 0
                sps = sp_pool.tile([128, 512], f32, tag="sps")
                nc.tensor.matmul(
                    sps[:, c0:],
                    lhsT=kT_sb[:, j * 128:(j + 1) * 128],
                    rhs=qT_sb[:, h, g * 512 + c0:(g + 1) * 512],
                    start=True, stop=True)
                P = pp.tile([128, 512], f16, tag="P")
                nc.scalar.activation(P[:, c0:], sps[:, c0:], Exp, scale=SCALE)
                if ing:
                    nc.vector.tensor_mul(
                        P[:, c0:c0 + 128], P[:, c0:c0 + 128], dmask[:])
                if j == 0:
                    nc.vector.tensor_copy(out=R[:], in_=P[:])
                else:
                    nc.vector.tensor_add(R[:, c0:], R[:, c0:], P[:, c0:])
                return P, c0

            def attn_pv(g, h, j, ops, P, c0):
                nc.tensor.matmul(
                    ops[:, c0:], lhsT=v_sb[:, j, :], rhs=P[:, c0:],
                    start=(j == 0), stop=(j == 4 * g + 3))

            def attn_tail(g, h, ops, R):
                # softmax denominators on Pool/DVE only; normalization is
                # fused into the outT eviction multiply
                gsl = slice(g * 512, (g + 1) * 512)
                l_bc = lp.tile([128, 512], f32, tag="lbc")
                nc.gpsimd.partition_all_reduce(
                    l_bc[:], R[:], channels=128,
                    reduce_op=bass_isa.ReduceOp.add)
                rbc = lp.tile([128, 512], f16, tag="rbc_sb")
                with nc.allow_low_precision(reason="1/l fits f16"):
                    nc.vector.reciprocal(rbc[:], l_bc[:])
                nc.vector.tensor_mul(outT_sb[:, h, gsl], ops[:], rbc[:])

            def attn_stream(groups, ops_pool, sp_pool):
                # generator, one yield per slot.  Each slot emits the PV of
                # the PREVIOUS j and the P-production of the current j, so
                # the PV always consumes a slot-old P (exp long finished)
                # and never blocks the PE wait queue.
                for g in groups:
                    for h in range(NQ):
                        ops = ops_pool.tile([128, 512], f32, tag="ops")
                        R = rr.tile([128, 512], f16, tag="R")
                        prev = None
                        for j in range(4 * g + 4):
                            if prev is not None:
                                attn_pv(g, h, j - 1, ops, *prev)
                            prev = attn_p(g, h, j, R, sp_pool)
                            yield
                        attn_pv(g, h, 4 * g + 3, ops, *prev)
                        attn_tail(g, h, ops, R)

            def pump(stream, n):
                for _ in range(n):
                    if next(stream, "done") == "done":
                        return

            # ---- supertiles 0..2: qkv + rope ----
            for st in range(3):
                xb = xbufs[st]
                sl = slice(st * 512, (st + 1) * 512)
                qpss = [ps1.tile([128, 512], f32, tag="qkv",
                                 name=f"qps{st}_{c6}") for c6 in range(5)]
                vt = vps.tile([128, 4, 128], f32, tag="vt", name=f"vps{st}")
                for kq in range(7):
                    for c6 in range(5):
                        for kc in range(kq * 4, kq * 4 + 4):
                            nc.tensor.matmul(
                                qpss[c6][:],
                                lhsT=w_sb[:, kc, c6 * 128:(c6 + 1) * 128],
                                rhs=xb[:, kc, :],
                                start=(kc == 0), stop=False)
                    for kc in range(kq * 4, kq * 4 + 4):
                        for q in range(4):
                            # start only on the first matmul into the bank:
                            # start=True zeroes the whole 2KB region, which
                            # pre-zeroes all four q-group slices
                            nc.tensor.matmul(
                                vt[:, q, :],
                                lhsT=xb[:, kc, q * 128:(q + 1) * 128],
                                rhs=w_sb[:, kc, 640:768],
                                start=(kc == 0 and q == 0), stop=False)
                for c6 in range(5):
                    for kc in range(28, 32):
                        nc.tensor.matmul(
                            qpss[c6][:],
                            lhsT=w_sb[:, kc, c6 * 128:(c6 + 1) * 128],
                            rhs=xb[:, kc, :],
                            start=False, stop=(kc == KC - 1))
                    evict_c6(st, c6, qpss[c6], sl)
                for kc in range(28, 32):
                    for q in range(4):
                        nc.tensor.matmul(
                            vt[:, q, :],
                            lhsT=xb[:, kc, q * 128:(q + 1) * 128],
                            rhs=w_sb[:, kc, 640:768],
                            start=False, stop=(kc == KC - 1))
                nc.scalar.copy(out=v_sb[:, st * 4:(st + 1) * 4, :],
                               in_=vt[:])

            # ---- overlap window: st3 qkv as sequential per-column streams,
            # g0+g1+g2 attention pumped between chunks ----
            ps1_cm.__exit__(None, None, None)
            ops2_cm = tc.tile_pool(name="ps_o2", bufs=2, space="PSUM")
            ops2 = ops2_cm.__enter__()
            ps1b_cm = tc.tile_pool(name="ps_qkv3", bufs=3, space="PSUM")
            ps1b = ps1b_cm.__enter__()

            g12_stream = attn_stream([0, 1, 2], ops2, ps_s)
            st, xb, sl = 3, xbufs[3], slice(1536, 2048)
            vt = vps.tile([128, 4, 128], f32, tag="vt", name="vps3")
            for c6 in range(5):
                qps = ps1b.tile([128, 512], f32, tag="qkv3")
                for k4 in range(0, KC, 4):
                    for kc in range(k4, k4 + 4):
                        nc.tensor.matmul(
                            qps[:],
                            lhsT=w_sb[:, kc, c6 * 128:(c6 + 1) * 128],
                            rhs=xb[:, kc, :],
                            start=(kc == 0), stop=(kc == KC - 1))
                    pump(g12_stream, 2)
                evict_c6(st, c6, qps, sl)
            for k4 in range(0, KC, 4):
                for kc in range(k4, k4 + 4):
                    for q in range(4):
                        nc.tensor.matmul(
                            vt[:, q, :],
                            lhsT=xb[:, kc, q * 128:(q + 1) * 128],
                            rhs=w_sb[:, kc, 640:768],
                            start=(kc == 0 and q == 0),
                            stop=(kc == KC - 1))
                pump(g12_stream, 2)
            nc.scalar.copy(out=v_sb[:, 12:16, :], in_=vt[:])
            pump(g12_stream, 999)  # drain leftover steps

            # ---- free phase-1 pools; g3 attention + all y tiles ----
            ps1b_cm.__exit__(None, None, None)
            rp_cm.__exit__(None, None, None)
            ep_cm.__exit__(None, None, None)
            csn_cm.__exit__(None, None, None)
            xb_pool_cm.__exit__(None, None, None)
            w_pool_cm.__exit__(None, None, None)

            yp_cm = tc.tile_pool(name="yp", bufs=6)
            yp = yp_cm.__enter__()
            ps_y_cm = tc.tile_pool(name="ps_y", bufs=3, space="PSUM")
            ps_y = ps_y_cm.__enter__()

            ycnt = 0

            def y_tile(gy, ym):
                nonlocal ycnt
                gysl = slice(gy * 512, (gy + 1) * 512)
                yps = ps_y.tile([128, 512], f32, tag="yps")
                for kc in range(NQ):
                    nc.tensor.matmul(
                        yps[:],
                        lhsT=wo_sb[:, kc, ym * 128:(ym + 1) * 128],
                        rhs=outT_sb[:, kc, gysl],
                        start=(kc == 0), stop=(kc == NQ - 1))
                ysm = yp.tile([128, 512], f16, tag="ysm")
                if ycnt % 2 == 0:
                    nc.scalar.copy(out=ysm[:], in_=yps[:])
                else:
                    nc.vector.tensor_copy(out=ysm[:], in_=yps[:])
                nc.sync.dma_start(out=yT_v[:, ym:ym + 1, gysl], in_=ysm[:])
                ycnt += 1

            yq = [(g, ym) for g in range(3) for ym in range(32)]
            yi = 0

            def pump_y(n):
                nonlocal yi
                for _ in range(n):
                    if yi < len(yq):
                        y_tile(*yq[yi])
                        yi += 1

            g3_stream = attn_stream([3], ops2, ps_s)
            g3i = 0
            while next(g3_stream, "done") != "done":
                pump_y(1 if g3i % 2 == 0 else 2)
                g3i += 1
            yq.extend((3, ym) for ym in range(32))
            pump_y(len(yq))

            ps_y_cm.__exit__(None, None, None)
            yp_cm.__exit__(None, None, None)
            ops2_cm.__exit__(None, None, None)
            vps_cm.__exit__(None, None, None)

    nc.compile()
    return nc


def _get_nc():
    if "nc" not in _CACHE:
        _CACHE["nc"] = _build()
    return _CACHE["nc"]


def _prep_inputs(x, rope_cache, wqkv, wo):
    x2 = np.asarray(x, np.float32).reshape(S, H)
    xT = np.ascontiguousarray(x2.T.astype(np.float16))          # [H, S]

    rc = np.asarray(rope_cache, np.float32)                      # [S, 64, 2]
    c = rc[:, :, 0].T.astype(np.float16)                         # [64, S]
    s = rc[:, :, 1].T.astype(np.float16)
    cs2 = np.ascontiguousarray(np.vstack([c, c]))                # [128, S]
    sn2 = np.ascontiguousarray(np.vstack([-s, s]))

    # transposed causal mask (multiplicative): keep k <= q
    kk = np.arange(128)
    dmask = (kk[:, None] <= kk[None, :]).astype(np.float16)

    # rope pair permutation within each 128-wide head: evens then odds
    perm = np.concatenate([np.arange(0, 128, 2), np.arange(1, 128, 2)])

    wq = np.asarray(wqkv, np.float32)
    wo_f = np.asarray(wo, np.float32)

    in_maps = []
    for cid in range(N_CORES):
        cols = []
        for hh in range(NQ):
            qh = wq[:, cid * 512 + hh * 128: cid * 512 + (hh + 1) * 128]
            cols.append(qh[:, perm])
        kh = wq[:, H + cid * 128: H + (cid + 1) * 128]
        cols.append(kh[:, perm])
        vh = wq[:, H + 1024 + cid * 128: H + 1024 + (cid + 1) * 128]
        cols.append(vh)
        wcat = np.concatenate(cols, axis=1).astype(np.float16)
        in_maps.append({
            "xT": xT,
            "w": np.ascontiguousarray(wcat),
            "wo": np.ascontiguousarray(
                wo_f[cid * 512:(cid + 1) * 512, :].astype(np.float16)),
            "cs2": cs2,
            "sn2": sn2,
            "dmask": dmask,
        })
    return in_maps


def kernel(x, last_pos, mask, rope_cache, wqkv, wo):
    global LAST_RESULTS
    from concourse.bass_utils import run_bass_kernel_spmd

    nc = _get_nc()
    in_maps = _prep_inputs(x, rope_cache, wqkv, wo)

    res = run_bass_kernel_spmd(nc, in_maps, list(range(N_CORES)))
    LAST_RESULTS = res
    if res.exec_time_ns is not None:
        print(f"HW exec time: {res.exec_time_ns} ns")
    yT = res.results[0]["yT"].astype(np.float64)
    for c in range(1, N_CORES):
        yT = yT + res.results[c]["yT"]
    return np.ascontiguousarray(yT.T).reshape(1, S, H).astype(np.float32)


# revision 4
# speedup vs baseline: 1.0037x; 1.0014x over previous
"""Llama3 attention prefill kernel for 8 Trainium2 NeuronCores.

Sharding: tensor-parallel over heads. Core c owns Q heads 4c..4c+3 and KV
head c (GQA group), plus the matching wqkv columns / wo rows. Each core
computes a partial output y_c = attn_c @ wo_c; the host sums the partials.

Schedule (single TileContext, one long PE stream with software pipelining):
  0. PE warm-up dummies at t=0 ride out the 0.65->2.4 GHz DVFS ramp while
     the first w/x DMA chunks land.
  1. Supertiles st=0..2: qkvT = w^T x in transposed layout (q/k) plus v in
     natural [pos, d] layout via lhsT=x; eager per-column PSUM eviction +
     RoPE on DVE.  g=0's flash-attention steps interleave into st=2.
  2. Overlap window: st=3's qkv runs as sequential per-column streams
     (2 PSUM banks) while g=1 and g=2 attention steps pump between chunks.
  3. g=3 attention + all y^T = wo^T out^T tiles interleaved; the kernel
     tail is a single y tile evict+DMA.
  Attention: S^T = K_j^T Q_g per (k-block, 512-wide q group) -> exp gives
  P^T directly, causal diagonal via multiplicative mask, row sums on
  gpsimd, normalization fused into the outT eviction multiply.
"""

import os
import sys

for _p in ("/opt/trn_rl_repo", "/root/.axon_site/_ro/trn_rl_repo"):
    if os.path.isdir(_p) and _p not in sys.path:
        sys.path.insert(0, _p)

import numpy as np

S = 2048
H = 4096
HD = 128
NQ = 4            # q heads per core
MQKV = 768        # per-core qkv columns: 512 q + 128 k + 128 v
N_CORES = 8
KC = H // 128     # 32 contraction chunks for qkv
KT = S // 128     # 16 pos tiles
NG = S // 512     # 4 q groups of 512 positions
SCALE = 1.0 / float(np.sqrt(HD))

_CACHE = {}
LAST_RESULTS = None


def _build():
    import concourse.tile as tile
    from concourse import bacc, bass_isa, mybir

    f32 = mybir.dt.float32
    f16 = mybir.dt.float16
    Exp = mybir.ActivationFunctionType.Exp

    nc = bacc.Bacc("TRN2", target_bir_lowering=False, debug=False)

    xT_ap = nc.dram_tensor("xT", [H, S], f16, kind="ExternalInput").ap()
    w_ap = nc.dram_tensor("w", [H, MQKV], f16, kind="ExternalInput").ap()
    wo_ap = nc.dram_tensor("wo", [NQ * HD, H], f16, kind="ExternalInput").ap()
    cs_ap = nc.dram_tensor("cs2", [128, S], f16, kind="ExternalInput").ap()
    sn_ap = nc.dram_tensor("sn2", [128, S], f16, kind="ExternalInput").ap()
    dm_ap = nc.dram_tensor("dmask", [128, 128], f16, kind="ExternalInput").ap()
    yT_ap = nc.dram_tensor("yT", [H, S], f16, kind="ExternalOutput").ap()

    # DRAM views for batched DMA: [p, chunk, col]
    xT_v = xT_ap.rearrange("(a p) s -> p a s", p=128)      # [128, 32, 2048]
    w_v = w_ap.rearrange("(a p) c -> p a c", p=128)        # [128, 32, 768]
    wo_v = wo_ap.rearrange("(a p) c -> p a c", p=128)      # [128, 4, 4096]
    yT_v = yT_ap.rearrange("(a p) s -> p a s", p=128)      # [128, 32, 2048]

    with tile.TileContext(nc) as tc:
        from contextlib import ExitStack

        with ExitStack() as ctx:
            const = ctx.enter_context(tc.tile_pool(name="const", bufs=1))
            dmask = const.tile([128, 128], f16)
            # warm-up dummies read dmask before its DMA lands (WAR is
            # sequenced by the framework); contents are irrelevant.
            nc.gpsimd.memset(dmask[:], 0.0)

            # resident tensors
            res = ctx.enter_context(tc.tile_pool(name="res", bufs=1))
            qT_sb = res.tile([128, NQ, S], f16, name="qT_sb")
            kT_sb = res.tile([128, S], f16, name="kT_sb")
            v_sb = res.tile([128, KT, 128], f16, name="v_sb")
            outT_sb = res.tile([128, NQ, S], f16, name="outT_sb")

            # wo resident from the start; DMA'd during phase 1.
            # Right-side stack order: pools that die mid-program (w) sit on
            # top of the program-lifetime attention pools (pool frees are
            # LIFO per side).
            wo_pool = ctx.enter_context(
                tc.tile_pool(name="wo_pool", bufs=1, side="right"))
            wo_sb = wo_pool.tile([128, NQ, H], f16, name="wo_sb")

            # attention SBUF pools (live from st2 through the end)
            pp = ctx.enter_context(
                tc.tile_pool(name="pp", bufs=3, side="right"))
            rr = ctx.enter_context(
                tc.tile_pool(name="rr", bufs=2, side="right"))
            lp = ctx.enter_context(
                tc.tile_pool(name="lp", bufs=1, side="right"))

            w_pool_cm = tc.tile_pool(name="w_pool", bufs=1, side="right")
            w_pool = w_pool_cm.__enter__()
            w_sb = w_pool.tile([128, KC, MQKV], f16, name="w_sb")

            xb_pool_cm = tc.tile_pool(name="xb", bufs=2)
            xb_pool = xb_pool_cm.__enter__()

            # rope streaming pools (cs/sn stream per supertile: 2-deep)
            csn_cm = tc.tile_pool(name="csn", bufs=2)
            csn = csn_cm.__enter__()
            ep_cm = tc.tile_pool(name="ep", bufs=3)
            ep = ep_cm.__enter__()
            rp_cm = tc.tile_pool(name="rp", bufs=2)
            rp = rp_cm.__enter__()

            # PSUM: scores(2) + v(1) + qkv(5) = 8 during st0-2
            ps_s = ctx.enter_context(
                tc.tile_pool(name="ps_s", bufs=2, space="PSUM"))
            vps_cm = tc.tile_pool(name="ps_v", bufs=1, space="PSUM")
            vps = vps_cm.__enter__()
            ps1_cm = tc.tile_pool(name="ps_qkv", bufs=5, space="PSUM")
            ps1 = ps1_cm.__enter__()

            # PE warm-up: the tensor engine ramps 0.65->1.2->2.4 GHz over
            # ~3us of continuous execution.  Dummy matmuls that depend only
            # on the memset ride out the ramp while the first w/x DMA
            # chunks are in flight; results are never read.
            wps = vps.tile([128, 4, 128], f32, tag="vt", name="warm_ps")
            for wi in range(28):
                nc.tensor.matmul(wps[:, 0, :], lhsT=dmask[:], rhs=dmask[:],
                                 start=True, stop=True)

            # ---- DMA schedule (single shared engine; order = priority) ----
            xbufs = [xb_pool.tile([128, KC, 512], f16, tag="xb",
                                  name=f"xb{st}") for st in range(NG)]
            _edges = [0, 1, 2, 4, 6, 8, 12, 16, 20, 24, 28, 32]
            for _a, _b in zip(_edges[:-1], _edges[1:]):
                ksl = slice(_a, _b)
                nc.sync.dma_start(out=w_sb[:, ksl, :], in_=w_v[:, ksl, :])
                nc.sync.dma_start(out=xbufs[0][:, ksl, :],
                                  in_=xT_v[:, ksl, 0:512])
            for k8 in range(0, KC, 8):
                nc.sync.dma_start(out=xbufs[1][:, k8:k8 + 8, :],
                                  in_=xT_v[:, k8:k8 + 8, 512:1024])
            cs_ts, sn_ts = [], []
            for st in range(NG):
                cs_ts.append(csn.tile([128, 512], f16, tag="cs",
                                      name=f"cs{st}"))
                sn_ts.append(csn.tile([128, 512], f16, tag="sn",
                                      name=f"sn{st}"))
            stsl = [slice(st * 512, (st + 1) * 512) for st in range(NG)]
            # cs/sn for st0/st1 land before their rope; st2/st3 slices reuse
            # the buffers, so their DMAs (which wait on st0/st1 rope) are
            # queued last to keep the in-order DMA queue from head-blocking
            for st in (0, 1):
                nc.sync.dma_start(out=cs_ts[st][:], in_=cs_ap[:, stsl[st]])
                nc.sync.dma_start(out=sn_ts[st][:], in_=sn_ap[:, stsl[st]])
            nc.sync.dma_start(out=dmask[:], in_=dm_ap[:, :])
            nc.sync.dma_start(out=xbufs[2][:], in_=xT_v[:, :, 1024:1536])
            nc.sync.dma_start(out=xbufs[3][:], in_=xT_v[:, :, 1536:2048])
            nc.sync.dma_start(out=wo_sb[:], in_=wo_v[:, :, :])
            for st in (2, 3):
                nc.sync.dma_start(out=cs_ts[st][:], in_=cs_ap[:, stsl[st]])
                nc.sync.dma_start(out=sn_ts[st][:], in_=sn_ap[:, stsl[st]])

            # ---- helpers ----
            def evict_c6(st, c6, qps, sl):
                # rope: partner half via two half-partition copies
                E = ep.tile([128, 512], f16, tag="E")
                nc.scalar.copy(out=E[:], in_=qps[:])
                Esw = ep.tile([128, 512], f16, tag="Esw")
                nc.vector.tensor_copy(out=Esw[0:64, :], in_=E[64:128, :])
                nc.vector.tensor_copy(out=Esw[64:128, :], in_=E[0:64, :])
                t1 = rp.tile([128, 512], f16, tag="t1")
                nc.vector.tensor_mul(t1[:], E[:], cs_ts[st][:])
                t2 = rp.tile([128, 512], f16, tag="t2")
                nc.vector.tensor_mul(t2[:], Esw[:], sn_ts[st][:])
                dst = (qT_sb[:, c6, sl] if c6 < 4 else kT_sb[:, sl])
                nc.vector.tensor_add(dst, t1[:], t2[:])

            def attn_p(g, h, j, R, sp_pool):
                # P production: S^T matmul -> exp -> causal mask -> R add
                ing = (j // 4 == g)
                c0 = (j - 4 * g) * 128 if ing else 0
                sps = sp_pool.tile([128, 512], f32, tag="sps")
                nc.tensor.matmul(
                    sps[:, c0:],
                    lhsT=kT_sb[:, j * 128:(j + 1) * 128],
                    rhs=qT_sb[:, h, g * 512 + c0:(g + 1) * 512],
                    start=True, stop=True)
                P = pp.tile([128, 512], f16, tag="P")
                nc.scalar.activation(P[:, c0:], sps[:, c0:], Exp, scale=SCALE)
                if ing:
                    nc.vector.tensor_mul(
                        P[:, c0:c0 + 128], P[:, c0:c0 + 128], dmask[:])
                if j == 0:
                    nc.vector.tensor_copy(out=R[:], in_=P[:])
                else:
                    nc.vector.tensor_add(R[:, c0:], R[:, c0:], P[:, c0:])
                return P, c0

            def attn_pv(g, h, j, ops, P, c0):
                nc.tensor.matmul(
                    ops[:, c0:], lhsT=v_sb[:, j, :], rhs=P[:, c0:],
                    start=(j == 0), stop=(j == 4 * g + 3))

            def attn_tail(g, h, ops, R):
                # softmax denominators on Pool/DVE only; normalization is
                # fused into the outT eviction multiply
                gsl = slice(g * 512, (g + 1) * 512)
                l_bc = lp.tile([128, 512], f32, tag="lbc")
                nc.gpsimd.partition_all_reduce(
                    l_bc[:], R[:], channels=128,
                    reduce_op=bass_isa.ReduceOp.add)
                rbc = lp.tile([128, 512], f16, tag="rbc_sb")
                with nc.allow_low_precision(reason="1/l fits f16"):
                    nc.vector.reciprocal(rbc[:], l_bc[:])
                nc.vector.tensor_mul(outT_sb[:, h, gsl], ops[:], rbc[:])

            def attn_stream(groups, ops_pool, sp_pool):
                # generator, one yield per slot.  Each slot emits the PV of
                # the PREVIOUS j and the P-production of the current j, so
                # the PV always consumes a slot-old P (exp long finished)
                # and never blocks the PE wait queue.
                for g in groups:
                    for h in range(NQ):
                        ops = ops_pool.tile([128, 512], f32, tag="ops")
                        R = rr.tile([128, 512], f16, tag="R")
                        prev = None
                        for j in range(4 * g + 4):
                            if prev is not None:
                                attn_pv(g, h, j - 1, ops, *prev)
                            prev = attn_p(g, h, j, R, sp_pool)
                            yield
                        attn_pv(g, h, 4 * g + 3, ops, *prev)
                        attn_tail(g, h, ops, R)

            def pump(stream, n):
                for _ in range(n):
                    if next(stream, "done") == "done":
                        return

            # ---- supertiles 0..2: qkv + rope ----
            for st in range(3):
                xb = xbufs[st]
                sl = slice(st * 512, (st + 1) * 512)
                qpss = [ps1.tile([128, 512], f32, tag="qkv",
                                 name=f"qps{st}_{c6}") for c6 in range(5)]
                vt = vps.tile([128, 4, 128], f32, tag="vt", name=f"vps{st}")
                for kq in range(7):
                    for c6 in range(5):
                        for kc in range(kq * 4, kq * 4 + 4):
                            nc.tensor.matmul(
                                qpss[c6][:],
                                lhsT=w_sb[:, kc, c6 * 128:(c6 + 1) * 128],
                                rhs=xb[:, kc, :],
                                start=(kc == 0), stop=False)
                    for kc in range(kq * 4, kq * 4 + 4):
                        for q in range(4):
                            # start only on the first matmul into the bank:
                            # start=True zeroes the whole 2KB region, which
                            # pre-zeroes all four q-group slices
                            nc.tensor.matmul(
                                vt[:, q, :],
                                lhsT=xb[:, kc, q * 128:(q + 1) * 128],
                                rhs=w_sb[:, kc, 640:768],
                                start=(kc == 0 and q == 0), stop=False)
                for c6 in range(5):
                    for kc in range(28, 32):
                        nc.tensor.matmul(
                            qpss[c6][:],
                            lhsT=w_sb[:, kc, c6 * 128:(c6 + 1) * 128],
                            rhs=xb[:, kc, :],
                            start=False, stop=(kc == KC - 1))
                    evict_c6(st, c6, qpss[c6], sl)
                for kc in range(28, 32):
                    for q in range(4):
                        nc.tensor.matmul(
                            vt[:, q, :],
                            lhsT=xb[:, kc, q * 128:(q + 1) * 128],
                            rhs=w_sb[:, kc, 640:768],
                            start=False, stop=(kc == KC - 1))
                nc.scalar.copy(out=v_sb[:, st * 4:(st + 1) * 4, :],
                               in_=vt[:])

            # ---- overlap window: st3 qkv as sequential per-column streams,
            # g0+g1+g2 attention pumped between chunks ----
            ps1_cm.__exit__(None, None, None)
            ops2_cm = tc.tile_pool(name="ps_o2", bufs=2, space="PSUM")
            ops2 = ops2_cm.__enter__()
            ps1b_cm = tc.tile_pool(name="ps_qkv3", bufs=3, space="PSUM")
            ps1b = ps1b_cm.__enter__()

            g12_stream = attn_stream([0, 1, 2], ops2, ps_s)
            st, xb, sl = 3, xbufs[3], slice(1536, 2048)
            vt = vps.tile([128, 4, 128], f32, tag="vt", name="vps3")
            for c6 in range(5):
                qps = ps1b.tile([128, 512], f32, tag="qkv3")
                for k4 in range(0, KC, 4):
                    for kc in range(k4, k4 + 4):
                        nc.tensor.matmul(
                            qps[:],
                            lhsT=w_sb[:, kc, c6 * 128:(c6 + 1) * 128],
                            rhs=xb[:, kc, :],
                            start=(kc == 0), stop=(kc == KC - 1))
                    pump(g12_stream, 2)
                evict_c6(st, c6, qps, sl)
            for k4 in range(0, KC, 4):
                for kc in range(k4, k4 + 4):
                    for q in range(4):
                        nc.tensor.matmul(
                            vt[:, q, :],
                            lhsT=xb[:, kc, q * 128:(q + 1) * 128],
                            rhs=w_sb[:, kc, 640:768],
                            start=(kc == 0 and q == 0),
                            stop=(kc == KC - 1))
                pump(g12_stream, 2)
            nc.scalar.copy(out=v_sb[:, 12:16, :], in_=vt[:])
            pump(g12_stream, 999)  # drain leftover steps

            # ---- free phase-1 pools; g3 attention + all y tiles ----
            ps1b_cm.__exit__(None, None, None)
            rp_cm.__exit__(None, None, None)
            ep_cm.__exit__(None, None, None)
            csn_cm.__exit__(None, None, None)
            xb_pool_cm.__exit__(None, None, None)
            w_pool_cm.__exit__(None, None, None)

            yp_cm = tc.tile_pool(name="yp", bufs=8)
            yp = yp_cm.__enter__()
            ps_y_cm = tc.tile_pool(name="ps_y", bufs=3, space="PSUM")
            ps_y = ps_y_cm.__enter__()

            ycnt = 0

            def y_tile(gy, ym):
                nonlocal ycnt
                gysl = slice(gy * 512, (gy + 1) * 512)
                yps = ps_y.tile([128, 512], f32, tag="yps")
                for kc in range(NQ):
                    nc.tensor.matmul(
                        yps[:],
                        lhsT=wo_sb[:, kc, ym * 128:(ym + 1) * 128],
                        rhs=outT_sb[:, kc, gysl],
                        start=(kc == 0), stop=(kc == NQ - 1))
                ysm = yp.tile([128, 512], f16, tag="ysm")
                if ycnt % 2 == 0:
                    nc.scalar.copy(out=ysm[:], in_=yps[:])
                else:
                    nc.vector.tensor_copy(out=ysm[:], in_=yps[:])
                nc.sync.dma_start(out=yT_v[:, ym:ym + 1, gysl], in_=ysm[:])
                ycnt += 1

            yq = [(g, ym) for g in range(3) for ym in range(32)]
            yi = 0

            def pump_y(n):
                nonlocal yi
                for _ in range(n):
                    if yi < len(yq):
                        y_tile(*yq[yi])
                        yi += 1

            g3_stream = attn_stream([3], ops2, ps_s)
            g3i = 0
            while next(g3_stream, "done") != "done":
                pump_y(1 if g3i % 2 == 0 else 2)
                g3i += 1
            yq.extend((3, ym) for ym in range(32))
            pump_y(len(yq))

            ps_y_cm.__exit__(None, None, None)
            yp_cm.__exit__(None, None, None)
            ops2_cm.__exit__(None, None, None)
            vps_cm.__exit__(None, None, None)

    nc.compile()
    return nc


def _get_nc():
    if "nc" not in _CACHE:
        _CACHE["nc"] = _build()
    return _CACHE["nc"]


def _prep_inputs(x, rope_cache, wqkv, wo):
    x2 = np.asarray(x, np.float32).reshape(S, H)
    xT = np.ascontiguousarray(x2.T.astype(np.float16))          # [H, S]

    rc = np.asarray(rope_cache, np.float32)                      # [S, 64, 2]
    c = rc[:, :, 0].T.astype(np.float16)                         # [64, S]
    s = rc[:, :, 1].T.astype(np.float16)
    cs2 = np.ascontiguousarray(np.vstack([c, c]))                # [128, S]
    sn2 = np.ascontiguousarray(np.vstack([-s, s]))

    # transposed causal mask (multiplicative): keep k <= q
    kk = np.arange(128)
    dmask = (kk[:, None] <= kk[None, :]).astype(np.float16)

    # rope pair permutation within each 128-wide head: evens then odds
    perm = np.concatenate([np.arange(0, 128, 2), np.arange(1, 128, 2)])

    wq = np.asarray(wqkv, np.float32)
    wo_f = np.asarray(wo, np.float32)

    in_maps = []
    for cid in range(N_CORES):
        cols = []
        for hh in range(NQ):
            qh = wq[:, cid * 512 + hh * 128: cid * 512 + (hh + 1) * 128]
            cols.append(qh[:, perm])
        kh = wq[:, H + cid * 128: H + (cid + 1) * 128]
        cols.append(kh[:, perm])
        vh = wq[:, H + 1024 + cid * 128: H + 1024 + (cid + 1) * 128]
        cols.append(vh)
        wcat = np.concatenate(cols, axis=1).astype(np.float16)
        in_maps.append({
            "xT": xT,
            "w": np.ascontiguousarray(wcat),
            "wo": np.ascontiguousarray(
                wo_f[cid * 512:(cid + 1) * 512, :].astype(np.float16)),
            "cs2": cs2,
            "sn2": sn2,
            "dmask": dmask,
        })
    return in_maps


def kernel(x, last_pos, mask, rope_cache, wqkv, wo):
    global LAST_RESULTS
    from concourse.bass_utils import run_bass_kernel_spmd

    nc = _get_nc()
    in_maps = _prep_inputs(x, rope_cache, wqkv, wo)

    res = run_bass_kernel_spmd(nc, in_maps, list(range(N_CORES)))
    LAST_RESULTS = res
    if res.exec_time_ns is not None:
        print(f"HW exec time: {res.exec_time_ns} ns")
    yT = res.results[0]["yT"].astype(np.float64)
    for c in range(1, N_CORES):
        yT = yT + res.results[c]["yT"]
    return np.ascontiguousarray(yT.T).reshape(1, S, H).astype(np.float32)


# revision 5
# speedup vs baseline: 1.0063x; 1.0026x over previous
"""Llama3 attention prefill kernel for 8 Trainium2 NeuronCores.

Sharding: tensor-parallel over heads. Core c owns Q heads 4c..4c+3 and KV
head c (GQA group), plus the matching wqkv columns / wo rows. Each core
computes a partial output y_c = attn_c @ wo_c; the host sums the partials.

Schedule (single TileContext, one long PE stream with software pipelining):
  0. PE warm-up dummies at t=0 ride out the 0.65->2.4 GHz DVFS ramp while
     the first w/x DMA chunks land.
  1. Supertiles st=0..2: qkvT = w^T x in transposed layout (q/k) plus v in
     natural [pos, d] layout via lhsT=x; eager per-column PSUM eviction +
     RoPE on DVE.  g=0's flash-attention steps interleave into st=2.
  2. Overlap window: st=3's qkv runs as sequential per-column streams
     (2 PSUM banks) while g=1 and g=2 attention steps pump between chunks.
  3. g=3 attention + all y^T = wo^T out^T tiles interleaved; the kernel
     tail is a single y tile evict+DMA.
  Attention: S^T = K_j^T Q_g per (k-block, 512-wide q group) -> exp gives
  P^T directly, causal diagonal via multiplicative mask, row sums on
  gpsimd, normalization fused into the outT eviction multiply.
"""

import os
import sys

for _p in ("/opt/trn_rl_repo", "/root/.axon_site/_ro/trn_rl_repo"):
    if os.path.isdir(_p) and _p not in sys.path:
        sys.path.insert(0, _p)

import numpy as np

S = 2048
H = 4096
HD = 128
NQ = 4            # q heads per core
MQKV = 768        # per-core qkv columns: 512 q + 128 k + 128 v
N_CORES = 8
KC = H // 128     # 32 contraction chunks for qkv
KT = S // 128     # 16 pos tiles
NG = S // 512     # 4 q groups of 512 positions
SCALE = 1.0 / float(np.sqrt(HD))

_CACHE = {}
LAST_RESULTS = None


def _build():
    import concourse.tile as tile
    from concourse import bacc, bass_isa, mybir

    f32 = mybir.dt.float32
    f16 = mybir.dt.float16
    Exp = mybir.ActivationFunctionType.Exp

    nc = bacc.Bacc("TRN2", target_bir_lowering=False, debug=False)

    xT_ap = nc.dram_tensor("xT", [H, S], f16, kind="ExternalInput").ap()
    w_ap = nc.dram_tensor("w", [H, MQKV], f16, kind="ExternalInput").ap()
    wo_ap = nc.dram_tensor("wo", [NQ * HD, H], f16, kind="ExternalInput").ap()
    cs_ap = nc.dram_tensor("cs2", [128, S], f16, kind="ExternalInput").ap()
    sn_ap = nc.dram_tensor("sn2", [128, S], f16, kind="ExternalInput").ap()
    dm_ap = nc.dram_tensor("dmask", [128, 128], f16, kind="ExternalInput").ap()
    yT_ap = nc.dram_tensor("yT", [H, S], f16, kind="ExternalOutput").ap()

    # DRAM views for batched DMA: [p, chunk, col]
    xT_v = xT_ap.rearrange("(a p) s -> p a s", p=128)      # [128, 32, 2048]
    w_v = w_ap.rearrange("(a p) c -> p a c", p=128)        # [128, 32, 768]
    wo_v = wo_ap.rearrange("(a p) c -> p a c", p=128)      # [128, 4, 4096]
    yT_v = yT_ap.rearrange("(a p) s -> p a s", p=128)      # [128, 32, 2048]

    with tile.TileContext(nc) as tc:
        from contextlib import ExitStack

        with ExitStack() as ctx:
            const = ctx.enter_context(tc.tile_pool(name="const", bufs=1))
            dmask = const.tile([128, 128], f16)
            # warm-up dummies read dmask before its DMA lands (WAR is
            # sequenced by the framework); contents are irrelevant.
            nc.gpsimd.memset(dmask[:], 0.0)

            # resident tensors
            res = ctx.enter_context(tc.tile_pool(name="res", bufs=1))
            qT_sb = res.tile([128, NQ, S], f16, name="qT_sb")
            kT_sb = res.tile([128, S], f16, name="kT_sb")
            v_sb = res.tile([128, KT, 128], f16, name="v_sb")
            outT_sb = res.tile([128, NQ, S], f16, name="outT_sb")

            # wo resident from the start; DMA'd during phase 1.
            # Right-side stack order: pools that die mid-program (w) sit on
            # top of the program-lifetime attention pools (pool frees are
            # LIFO per side).
            wo_pool = ctx.enter_context(
                tc.tile_pool(name="wo_pool", bufs=1, side="right"))
            wo_sb = wo_pool.tile([128, NQ, H], f16, name="wo_sb")

            # attention SBUF pools (live from st2 through the end)
            pp = ctx.enter_context(
                tc.tile_pool(name="pp", bufs=3, side="right"))
            rr = ctx.enter_context(
                tc.tile_pool(name="rr", bufs=2, side="right"))
            lp = ctx.enter_context(
                tc.tile_pool(name="lp", bufs=1, side="right"))

            w_pool_cm = tc.tile_pool(name="w_pool", bufs=1, side="right")
            w_pool = w_pool_cm.__enter__()
            w_sb = w_pool.tile([128, KC, MQKV], f16, name="w_sb")

            xb_pool_cm = tc.tile_pool(name="xb", bufs=2)
            xb_pool = xb_pool_cm.__enter__()

            # rope streaming pools (cs/sn stream per supertile: 2-deep)
            csn_cm = tc.tile_pool(name="csn", bufs=2)
            csn = csn_cm.__enter__()
            ep_cm = tc.tile_pool(name="ep", bufs=3)
            ep = ep_cm.__enter__()
            rp_cm = tc.tile_pool(name="rp", bufs=2)
            rp = rp_cm.__enter__()

            # PSUM: scores(2) + v(1) + qkv(5) = 8 during st0-2
            ps_s = ctx.enter_context(
                tc.tile_pool(name="ps_s", bufs=2, space="PSUM"))
            vps_cm = tc.tile_pool(name="ps_v", bufs=1, space="PSUM")
            vps = vps_cm.__enter__()
            ps1_cm = tc.tile_pool(name="ps_qkv", bufs=5, space="PSUM")
            ps1 = ps1_cm.__enter__()

            # PE warm-up: the tensor engine ramps 0.65->1.2->2.4 GHz over
            # ~3us of continuous execution.  Dummy matmuls that depend only
            # on the memset ride out the ramp while the first w/x DMA
            # chunks are in flight; results are never read.
            wps = vps.tile([128, 4, 128], f32, tag="vt", name="warm_ps")
            for wi in range(28):
                nc.tensor.matmul(wps[:, 0, :], lhsT=dmask[:], rhs=dmask[:],
                                 start=True, stop=True)

            # ---- DMA schedule (single shared engine; order = priority) ----
            xbufs = [xb_pool.tile([128, KC, 512], f16, tag="xb",
                                  name=f"xb{st}") for st in range(NG)]
            _edges = [0, 1, 2, 4, 6, 8, 12, 16, 20, 24, 28, 32]
            for _a, _b in zip(_edges[:-1], _edges[1:]):
                ksl = slice(_a, _b)
                nc.sync.dma_start(out=w_sb[:, ksl, :], in_=w_v[:, ksl, :])
                nc.sync.dma_start(out=xbufs[0][:, ksl, :],
                                  in_=xT_v[:, ksl, 0:512])
            for k8 in range(0, KC, 8):
                nc.sync.dma_start(out=xbufs[1][:, k8:k8 + 8, :],
                                  in_=xT_v[:, k8:k8 + 8, 512:1024])
            cs_ts, sn_ts = [], []
            for st in range(NG):
                cs_ts.append(csn.tile([128, 512], f16, tag="cs",
                                      name=f"cs{st}"))
                sn_ts.append(csn.tile([128, 512], f16, tag="sn",
                                      name=f"sn{st}"))
            stsl = [slice(st * 512, (st + 1) * 512) for st in range(NG)]
            # cs/sn for st0/st1 land before their rope; st2/st3 slices reuse
            # the buffers, so their DMAs (which wait on st0/st1 rope) are
            # queued last to keep the in-order DMA queue from head-blocking
            for st in (0, 1):
                nc.sync.dma_start(out=cs_ts[st][:], in_=cs_ap[:, stsl[st]])
                nc.sync.dma_start(out=sn_ts[st][:], in_=sn_ap[:, stsl[st]])
            nc.sync.dma_start(out=dmask[:], in_=dm_ap[:, :])
            nc.sync.dma_start(out=xbufs[2][:], in_=xT_v[:, :, 1024:1536])
            nc.sync.dma_start(out=xbufs[3][:], in_=xT_v[:, :, 1536:2048])
            nc.sync.dma_start(out=wo_sb[:], in_=wo_v[:, :, :])
            for st in (2, 3):
                nc.sync.dma_start(out=cs_ts[st][:], in_=cs_ap[:, stsl[st]])
                nc.sync.dma_start(out=sn_ts[st][:], in_=sn_ap[:, stsl[st]])

            # ---- helpers ----
            def evict_c6(st, c6, qps, sl):
                # rope: partner half via two half-partition copies
                E = ep.tile([128, 512], f16, tag="E")
                nc.scalar.copy(out=E[:], in_=qps[:])
                Esw = ep.tile([128, 512], f16, tag="Esw")
                nc.vector.tensor_copy(out=Esw[0:64, :], in_=E[64:128, :])
                nc.vector.tensor_copy(out=Esw[64:128, :], in_=E[0:64, :])
                t1 = rp.tile([128, 512], f16, tag="t1")
                nc.vector.tensor_mul(t1[:], E[:], cs_ts[st][:])
                t2 = rp.tile([128, 512], f16, tag="t2")
                nc.vector.tensor_mul(t2[:], Esw[:], sn_ts[st][:])
                dst = (qT_sb[:, c6, sl] if c6 < 4 else kT_sb[:, sl])
                nc.vector.tensor_add(dst, t1[:], t2[:])

            def attn_p(g, h, j, R, sp_pool):
                # P production: S^T matmul -> exp -> causal mask -> R add
                ing = (j // 4 == g)
                c0 = (j - 4 * g) * 128 if ing else 0
                sps = sp_pool.tile([128, 512], f32, tag="sps")
                nc.tensor.matmul(
                    sps[:, c0:],
                    lhsT=kT_sb[:, j * 128:(j + 1) * 128],
                    rhs=qT_sb[:, h, g * 512 + c0:(g + 1) * 512],
                    start=True, stop=True)
                P = pp.tile([128, 512], f16, tag="P")
                nc.scalar.activation(P[:, c0:], sps[:, c0:], Exp, scale=SCALE)
                if ing:
                    nc.vector.tensor_mul(
                        P[:, c0:c0 + 128], P[:, c0:c0 + 128], dmask[:])
                if j == 0:
                    nc.vector.tensor_copy(out=R[:], in_=P[:])
                else:
                    nc.vector.tensor_add(R[:, c0:], R[:, c0:], P[:, c0:])
                return P, c0

            def attn_pv(g, h, j, ops, P, c0):
                nc.tensor.matmul(
                    ops[:, c0:], lhsT=v_sb[:, j, :], rhs=P[:, c0:],
                    start=(j == 0), stop=(j == 4 * g + 3))

            def attn_tail(g, h, ops, R):
                # softmax denominators on Pool/DVE only; normalization is
                # fused into the outT eviction multiply
                gsl = slice(g * 512, (g + 1) * 512)
                l_bc = lp.tile([128, 512], f32, tag="lbc")
                nc.gpsimd.partition_all_reduce(
                    l_bc[:], R[:], channels=128,
                    reduce_op=bass_isa.ReduceOp.add)
                rbc = lp.tile([128, 512], f16, tag="rbc_sb")
                with nc.allow_low_precision(reason="1/l fits f16"):
                    nc.vector.reciprocal(rbc[:], l_bc[:])
                nc.vector.tensor_mul(outT_sb[:, h, gsl], ops[:], rbc[:])

            def attn_stream(groups, ops_pool, sp_pool):
                # generator, one yield per slot.  Each slot emits the PV of
                # the PREVIOUS j and the P-production of the current j, so
                # the PV always consumes a slot-old P (exp long finished)
                # and never blocks the PE wait queue.
                for g in groups:
                    for h in range(NQ):
                        ops = ops_pool.tile([128, 512], f32, tag="ops")
                        R = rr.tile([128, 512], f16, tag="R")
                        prev = None
                        for j in range(4 * g + 4):
                            if prev is not None:
                                attn_pv(g, h, j - 1, ops, *prev)
                            prev = attn_p(g, h, j, R, sp_pool)
                            yield
                        attn_pv(g, h, 4 * g + 3, ops, *prev)
                        attn_tail(g, h, ops, R)

            def pump(stream, n):
                for _ in range(n):
                    if next(stream, "done") == "done":
                        return

            # ---- supertiles 0..2: qkv + rope ----
            for st in range(3):
                xb = xbufs[st]
                sl = slice(st * 512, (st + 1) * 512)
                qpss = [ps1.tile([128, 512], f32, tag="qkv",
                                 name=f"qps{st}_{c6}") for c6 in range(5)]
                vt = vps.tile([128, 4, 128], f32, tag="vt", name=f"vps{st}")
                for kq in range(7):
                    for c6 in range(5):
                        for kc in range(kq * 4, kq * 4 + 4):
                            nc.tensor.matmul(
                                qpss[c6][:],
                                lhsT=w_sb[:, kc, c6 * 128:(c6 + 1) * 128],
                                rhs=xb[:, kc, :],
                                start=(kc == 0), stop=False)
                    for kc in range(kq * 4, kq * 4 + 4):
                        for q in range(4):
                            # start only on the first matmul into the bank:
                            # start=True zeroes the whole 2KB region, which
                            # pre-zeroes all four q-group slices
                            nc.tensor.matmul(
                                vt[:, q, :],
                                lhsT=xb[:, kc, q * 128:(q + 1) * 128],
                                rhs=w_sb[:, kc, 640:768],
                                start=(kc == 0 and q == 0), stop=False)
                for c6 in range(5):
                    for kc in range(28, 32):
                        nc.tensor.matmul(
                            qpss[c6][:],
                            lhsT=w_sb[:, kc, c6 * 128:(c6 + 1) * 128],
                            rhs=xb[:, kc, :],
                            start=False, stop=(kc == KC - 1))
                    evict_c6(st, c6, qpss[c6], sl)
                for kc in range(28, 32):
                    for q in range(4):
                        nc.tensor.matmul(
                            vt[:, q, :],
                            lhsT=xb[:, kc, q * 128:(q + 1) * 128],
                            rhs=w_sb[:, kc, 640:768],
                            start=False, stop=(kc == KC - 1))
                nc.scalar.copy(out=v_sb[:, st * 4:(st + 1) * 4, :],
                               in_=vt[:])

            # ---- overlap window: st3 qkv as sequential per-column streams,
            # g0+g1+g2 attention pumped between chunks ----
            ps1_cm.__exit__(None, None, None)
            ops2_cm = tc.tile_pool(name="ps_o2", bufs=2, space="PSUM")
            ops2 = ops2_cm.__enter__()
            ps1b_cm = tc.tile_pool(name="ps_qkv3", bufs=3, space="PSUM")
            ps1b = ps1b_cm.__enter__()

            g12_stream = attn_stream([0, 1, 2], ops2, ps_s)
            st, xb, sl = 3, xbufs[3], slice(1536, 2048)
            vt = vps.tile([128, 4, 128], f32, tag="vt", name="vps3")
            for c6 in range(5):
                qps = ps1b.tile([128, 512], f32, tag="qkv3")
                for k4 in range(0, KC, 4):
                    for kc in range(k4, k4 + 4):
                        nc.tensor.matmul(
                            qps[:],
                            lhsT=w_sb[:, kc, c6 * 128:(c6 + 1) * 128],
                            rhs=xb[:, kc, :],
                            start=(kc == 0), stop=(kc == KC - 1))
                    pump(g12_stream, 2)
                evict_c6(st, c6, qps, sl)
            for k4 in range(0, KC, 4):
                for kc in range(k4, k4 + 4):
                    for q in range(4):
                        nc.tensor.matmul(
                            vt[:, q, :],
                            lhsT=xb[:, kc, q * 128:(q + 1) * 128],
                            rhs=w_sb[:, kc, 640:768],
                            start=(kc == 0 and q == 0),
                            stop=(kc == KC - 1))
                pump(g12_stream, 2)
            nc.scalar.copy(out=v_sb[:, 12:16, :], in_=vt[:])
            pump(g12_stream, 999)  # drain leftover steps

            # ---- free phase-1 pools; g3 attention + all y tiles ----
            ps1b_cm.__exit__(None, None, None)
            rp_cm.__exit__(None, None, None)
            ep_cm.__exit__(None, None, None)
            csn_cm.__exit__(None, None, None)
            xb_pool_cm.__exit__(None, None, None)
            w_pool_cm.__exit__(None, None, None)

            yp_cm = tc.tile_pool(name="yp", bufs=8)
            yp = yp_cm.__enter__()
            ps_y_cm = tc.tile_pool(name="ps_y", bufs=3, space="PSUM")
            ps_y = ps_y_cm.__enter__()

            ycnt = 0

            def y_tile(gy, ym):
                nonlocal ycnt
                gysl = slice(gy * 512, (gy + 1) * 512)
                yps = ps_y.tile([128, 512], f32, tag="yps")
                for kc in range(NQ):
                    nc.tensor.matmul(
                        yps[:],
                        lhsT=wo_sb[:, kc, ym * 128:(ym + 1) * 128],
                        rhs=outT_sb[:, kc, gysl],
                        start=(kc == 0), stop=(kc == NQ - 1))
                ysm = yp.tile([128, 512], f16, tag="ysm")
                if ycnt % 2 == 0:
                    nc.scalar.copy(out=ysm[:], in_=yps[:])
                else:
                    nc.vector.tensor_copy(out=ysm[:], in_=yps[:])
                nc.sync.dma_start(out=yT_v[:, ym:ym + 1, gysl], in_=ysm[:])
                ycnt += 1

            yq = [(g, ym) for g in range(3) for ym in range(32)]
            yi = 0

            def pump_y(n):
                nonlocal yi
                for _ in range(n):
                    if yi < len(yq):
                        y_tile(*yq[yi])
                        yi += 1

            g3_stream = attn_stream([3], ops2, ps_s)
            g3i = 0
            while next(g3_stream, "done") != "done":
                # slow the pump through the last head so a few non-g3 tiles
                # remain to bridge the final softmax-tail latency before the
                # y(3) tiles become available
                pump_y(1 if (g3i % 2 == 0 or g3i >= 48) else 2)
                g3i += 1
            yq.extend((3, ym) for ym in range(32))
            pump_y(len(yq))

            ps_y_cm.__exit__(None, None, None)
            yp_cm.__exit__(None, None, None)
            ops2_cm.__exit__(None, None, None)
            vps_cm.__exit__(None, None, None)

    nc.compile()
    return nc


def _get_nc():
    if "nc" not in _CACHE:
        _CACHE["nc"] = _build()
    return _CACHE["nc"]


def _prep_inputs(x, rope_cache, wqkv, wo):
    x2 = np.asarray(x, np.float32).reshape(S, H)
    xT = np.ascontiguousarray(x2.T.astype(np.float16))          # [H, S]

    rc = np.asarray(rope_cache, np.float32)                      # [S, 64, 2]
    c = rc[:, :, 0].T.astype(np.float16)                         # [64, S]
    s = rc[:, :, 1].T.astype(np.float16)
    cs2 = np.ascontiguousarray(np.vstack([c, c]))                # [128, S]
    sn2 = np.ascontiguousarray(np.vstack([-s, s]))

    # transposed causal mask (multiplicative): keep k <= q
    kk = np.arange(128)
    dmask = (kk[:, None] <= kk[None, :]).astype(np.float16)

    # rope pair permutation within each 128-wide head: evens then odds
    perm = np.concatenate([np.arange(0, 128, 2), np.arange(1, 128, 2)])

    wq = np.asarray(wqkv, np.float32)
    wo_f = np.asarray(wo, np.float32)

    in_maps = []
    for cid in range(N_CORES):
        cols = []
        for hh in range(NQ):
            qh = wq[:, cid * 512 + hh * 128: cid * 512 + (hh + 1) * 128]
            cols.append(qh[:, perm])
        kh = wq[:, H + cid * 128: H + (cid + 1) * 128]
        cols.append(kh[:, perm])
        vh = wq[:, H + 1024 + cid * 128: H + 1024 + (cid + 1) * 128]
        cols.append(vh)
        wcat = np.concatenate(cols, axis=1).astype(np.float16)
        in_maps.append({
            "xT": xT,
            "w": np.ascontiguousarray(wcat),
            "wo": np.ascontiguousarray(
                wo_f[cid * 512:(cid + 1) * 512, :].astype(np.float16)),
            "cs2": cs2,
            "sn2": sn2,
            "dmask": dmask,
        })
    return in_maps


def kernel(x, last_pos, mask, rope_cache, wqkv, wo):
    global LAST_RESULTS
    from concourse.bass_utils import run_bass_kernel_spmd

    nc = _get_nc()
    in_maps = _prep_inputs(x, rope_cache, wqkv, wo)

    res = run_bass_kernel_spmd(nc, in_maps, list(range(N_CORES)))
    LAST_RESULTS = res
    if res.exec_time_ns is not None:
        print(f"HW exec time: {res.exec_time_ns} ns")
    yT = res.results[0]["yT"].astype(np.float64)
    for c in range(1, N_CORES):
        yT = yT + res.results[c]["yT"]
    return np.ascontiguousarray(yT.T).reshape(1, S, H).astype(np.float32)


# revision 6
# speedup vs baseline: 1.0067x; 1.0004x over previous
"""Llama3 attention prefill kernel for 8 Trainium2 NeuronCores.

Sharding: tensor-parallel over heads. Core c owns Q heads 4c..4c+3 and KV
head c (GQA group), plus the matching wqkv columns / wo rows. Each core
computes a partial output y_c = attn_c @ wo_c; the host sums the partials.

Schedule (single TileContext, one long PE stream with software pipelining):
  0. PE warm-up dummies at t=0 ride out the 0.65->2.4 GHz DVFS ramp while
     the first w/x DMA chunks land.
  1. Supertiles st=0..2: qkvT = w^T x in transposed layout (q/k) plus v in
     natural [pos, d] layout via lhsT=x; eager per-column PSUM eviction +
     RoPE on DVE.  g=0's flash-attention steps interleave into st=2.
  2. Overlap window: st=3's qkv runs as sequential per-column streams
     (2 PSUM banks) while g=1 and g=2 attention steps pump between chunks.
  3. g=3 attention + all y^T = wo^T out^T tiles interleaved; the kernel
     tail is a single y tile evict+DMA.
  Attention: S^T = K_j^T Q_g per (k-block, 512-wide q group) -> exp gives
  P^T directly, causal diagonal via multiplicative mask, row sums on
  gpsimd, normalization fused into the outT eviction multiply.
"""

import os
import sys

for _p in ("/opt/trn_rl_repo", "/root/.axon_site/_ro/trn_rl_repo"):
    if os.path.isdir(_p) and _p not in sys.path:
        sys.path.insert(0, _p)

import numpy as np

S = 2048
H = 4096
HD = 128
NQ = 4            # q heads per core
MQKV = 768        # per-core qkv columns: 512 q + 128 k + 128 v
N_CORES = 8
KC = H // 128     # 32 contraction chunks for qkv
KT = S // 128     # 16 pos tiles
NG = S // 512     # 4 q groups of 512 positions
SCALE = 1.0 / float(np.sqrt(HD))

_CACHE = {}
LAST_RESULTS = None


def _build():
    import concourse.tile as tile
    from concourse import bacc, bass_isa, mybir

    f32 = mybir.dt.float32
    f16 = mybir.dt.float16
    Exp = mybir.ActivationFunctionType.Exp

    nc = bacc.Bacc("TRN2", target_bir_lowering=False, debug=False)

    xT_ap = nc.dram_tensor("xT", [H, S], f16, kind="ExternalInput").ap()
    w_ap = nc.dram_tensor("w", [H, MQKV], f16, kind="ExternalInput").ap()
    wo_ap = nc.dram_tensor("wo", [NQ * HD, H], f16, kind="ExternalInput").ap()
    cs_ap = nc.dram_tensor("cs2", [128, S], f16, kind="ExternalInput").ap()
    sn_ap = nc.dram_tensor("sn2", [128, S], f16, kind="ExternalInput").ap()
    dm_ap = nc.dram_tensor("dmask", [128, 128], f16, kind="ExternalInput").ap()
    yT_ap = nc.dram_tensor("yT", [H, S], f16, kind="ExternalOutput").ap()

    # DRAM views for batched DMA: [p, chunk, col]
    xT_v = xT_ap.rearrange("(a p) s -> p a s", p=128)      # [128, 32, 2048]
    w_v = w_ap.rearrange("(a p) c -> p a c", p=128)        # [128, 32, 768]
    wo_v = wo_ap.rearrange("(a p) c -> p a c", p=128)      # [128, 4, 4096]
    yT_v = yT_ap.rearrange("(a p) s -> p a s", p=128)      # [128, 32, 2048]

    with tile.TileContext(nc) as tc:
        from contextlib import ExitStack

        with ExitStack() as ctx:
            const = ctx.enter_context(tc.tile_pool(name="const", bufs=1))
            dmask = const.tile([128, 128], f16)
            # warm-up dummies read dmask before its DMA lands (WAR is
            # sequenced by the framework); contents are irrelevant.
            nc.gpsimd.memset(dmask[:], 0.0)

            # resident tensors
            res = ctx.enter_context(tc.tile_pool(name="res", bufs=1))
            qT_sb = res.tile([128, NQ, S], f16, name="qT_sb")
            kT_sb = res.tile([128, S], f16, name="kT_sb")
            v_sb = res.tile([128, KT, 128], f16, name="v_sb")
            outT_sb = res.tile([128, NQ, S], f16, name="outT_sb")

            # wo resident from the start; DMA'd during phase 1.
            # Right-side stack order: pools that die mid-program (w) sit on
            # top of the program-lifetime attention pools (pool frees are
            # LIFO per side).
            wo_pool = ctx.enter_context(
                tc.tile_pool(name="wo_pool", bufs=1, side="right"))
            wo_sb = wo_pool.tile([128, NQ, H], f16, name="wo_sb")

            # attention SBUF pools (live from st2 through the end)
            pp = ctx.enter_context(
                tc.tile_pool(name="pp", bufs=3, side="right"))
            rr = ctx.enter_context(
                tc.tile_pool(name="rr", bufs=2, side="right"))
            lp = ctx.enter_context(
                tc.tile_pool(name="lp", bufs=1, side="right"))

            w_pool_cm = tc.tile_pool(name="w_pool", bufs=1, side="right")
            w_pool = w_pool_cm.__enter__()
            w_sb = w_pool.tile([128, KC, MQKV], f16, name="w_sb")

            xb_pool_cm = tc.tile_pool(name="xb", bufs=2)
            xb_pool = xb_pool_cm.__enter__()

            # rope streaming pools (cs/sn stream per supertile: 2-deep)
            csn_cm = tc.tile_pool(name="csn", bufs=2)
            csn = csn_cm.__enter__()
            ep_cm = tc.tile_pool(name="ep", bufs=3)
            ep = ep_cm.__enter__()
            rp_cm = tc.tile_pool(name="rp", bufs=2)
            rp = rp_cm.__enter__()

            # PSUM: scores(2) + v(1) + qkv(5) = 8 during st0-2
            ps_s = ctx.enter_context(
                tc.tile_pool(name="ps_s", bufs=2, space="PSUM"))
            vps_cm = tc.tile_pool(name="ps_v", bufs=1, space="PSUM")
            vps = vps_cm.__enter__()
            ps1_cm = tc.tile_pool(name="ps_qkv", bufs=5, space="PSUM")
            ps1 = ps1_cm.__enter__()

            # PE warm-up: the tensor engine ramps 0.65->1.2->2.4 GHz over
            # ~3us of continuous execution.  Dummy matmuls that depend only
            # on the memset ride out the ramp while the first w/x DMA
            # chunks are in flight; results are never read.
            wps = vps.tile([128, 4, 128], f32, tag="vt", name="warm_ps")
            for wi in range(28):
                nc.tensor.matmul(wps[:, 0, :], lhsT=dmask[:], rhs=dmask[:],
                                 start=True, stop=True)

            # ---- DMA schedule (single shared engine; order = priority) ----
            xbufs = [xb_pool.tile([128, KC, 512], f16, tag="xb",
                                  name=f"xb{st}") for st in range(NG)]
            _edges = [0, 1, 2, 4, 6, 8, 12, 16, 20, 24, 28, 32]
            for _a, _b in zip(_edges[:-1], _edges[1:]):
                ksl = slice(_a, _b)
                nc.sync.dma_start(out=w_sb[:, ksl, :], in_=w_v[:, ksl, :])
                nc.sync.dma_start(out=xbufs[0][:, ksl, :],
                                  in_=xT_v[:, ksl, 0:512])
            for k8 in range(0, KC, 8):
                nc.sync.dma_start(out=xbufs[1][:, k8:k8 + 8, :],
                                  in_=xT_v[:, k8:k8 + 8, 512:1024])
            cs_ts, sn_ts = [], []
            for st in range(NG):
                cs_ts.append(csn.tile([128, 512], f16, tag="cs",
                                      name=f"cs{st}"))
                sn_ts.append(csn.tile([128, 512], f16, tag="sn",
                                      name=f"sn{st}"))
            stsl = [slice(st * 512, (st + 1) * 512) for st in range(NG)]
            # cs/sn for st0/st1 land before their rope; st2/st3 slices reuse
            # the buffers, so their DMAs (which wait on st0/st1 rope) are
            # queued last to keep the in-order DMA queue from head-blocking
            for st in (0, 1):
                nc.sync.dma_start(out=cs_ts[st][:], in_=cs_ap[:, stsl[st]])
                nc.sync.dma_start(out=sn_ts[st][:], in_=sn_ap[:, stsl[st]])
            nc.sync.dma_start(out=dmask[:], in_=dm_ap[:, :])
            nc.sync.dma_start(out=xbufs[2][:], in_=xT_v[:, :, 1024:1536])
            nc.sync.dma_start(out=xbufs[3][:], in_=xT_v[:, :, 1536:2048])
            nc.sync.dma_start(out=wo_sb[:], in_=wo_v[:, :, :])
            for st in (2, 3):
                nc.sync.dma_start(out=cs_ts[st][:], in_=cs_ap[:, stsl[st]])
                nc.sync.dma_start(out=sn_ts[st][:], in_=sn_ap[:, stsl[st]])

            # ---- helpers ----
            def evict_c6(st, c6, qps, sl):
                # rope: partner half via two half-partition copies
                E = ep.tile([128, 512], f16, tag="E")
                nc.scalar.copy(out=E[:], in_=qps[:])
                Esw = ep.tile([128, 512], f16, tag="Esw")
                nc.vector.tensor_copy(out=Esw[0:64, :], in_=E[64:128, :])
                nc.vector.tensor_copy(out=Esw[64:128, :], in_=E[0:64, :])
                t1 = rp.tile([128, 512], f16, tag="t1")
                nc.vector.tensor_mul(t1[:], E[:], cs_ts[st][:])
                t2 = rp.tile([128, 512], f16, tag="t2")
                nc.vector.tensor_mul(t2[:], Esw[:], sn_ts[st][:])
                dst = (qT_sb[:, c6, sl] if c6 < 4 else kT_sb[:, sl])
                nc.vector.tensor_add(dst, t1[:], t2[:])

            def attn_p(g, h, j, R, sp_pool):
                # P production: S^T matmul -> exp -> causal mask -> R add
                ing = (j // 4 == g)
                c0 = (j - 4 * g) * 128 if ing else 0
                sps = sp_pool.tile([128, 512], f32, tag="sps")
                nc.tensor.matmul(
                    sps[:, c0:],
                    lhsT=kT_sb[:, j * 128:(j + 1) * 128],
                    rhs=qT_sb[:, h, g * 512 + c0:(g + 1) * 512],
                    start=True, stop=True)
                P = pp.tile([128, 512], f16, tag="P")
                nc.scalar.activation(P[:, c0:], sps[:, c0:], Exp, scale=SCALE)
                if ing:
                    nc.vector.tensor_mul(
                        P[:, c0:c0 + 128], P[:, c0:c0 + 128], dmask[:])
                if j == 0:
                    nc.vector.tensor_copy(out=R[:], in_=P[:])
                else:
                    nc.vector.tensor_add(R[:, c0:], R[:, c0:], P[:, c0:])
                return P, c0

            def attn_pv(g, h, j, ops, P, c0):
                nc.tensor.matmul(
                    ops[:, c0:], lhsT=v_sb[:, j, :], rhs=P[:, c0:],
                    start=(j == 0), stop=(j == 4 * g + 3))

            def attn_tail(g, h, ops, R):
                # softmax denominators on Pool/DVE only; normalization is
                # fused into the outT eviction multiply
                gsl = slice(g * 512, (g + 1) * 512)
                l_bc = lp.tile([128, 512], f32, tag="lbc")
                nc.gpsimd.partition_all_reduce(
                    l_bc[:], R[:], channels=128,
                    reduce_op=bass_isa.ReduceOp.add)
                rbc = lp.tile([128, 512], f16, tag="rbc_sb")
                with nc.allow_low_precision(reason="1/l fits f16"):
                    nc.vector.reciprocal(rbc[:], l_bc[:])
                nc.vector.tensor_mul(outT_sb[:, h, gsl], ops[:], rbc[:])

            def attn_stream(groups, ops_pool, sp_pool):
                # generator, one yield per slot.  Each slot emits the PV of
                # the PREVIOUS j and the P-production of the current j, so
                # the PV always consumes a slot-old P (exp long finished)
                # and never blocks the PE wait queue.
                for g in groups:
                    for h in range(NQ):
                        ops = ops_pool.tile([128, 512], f32, tag="ops")
                        R = rr.tile([128, 512], f16, tag="R")
                        prev = None
                        for j in range(4 * g + 4):
                            if prev is not None:
                                attn_pv(g, h, j - 1, ops, *prev)
                            prev = attn_p(g, h, j, R, sp_pool)
                            yield
                        attn_pv(g, h, 4 * g + 3, ops, *prev)
                        attn_tail(g, h, ops, R)

            def pump(stream, n):
                for _ in range(n):
                    if next(stream, "done") == "done":
                        return

            # ---- supertiles 0..2: qkv + rope ----
            for st in range(3):
                xb = xbufs[st]
                sl = slice(st * 512, (st + 1) * 512)
                qpss = [ps1.tile([128, 512], f32, tag="qkv",
                                 name=f"qps{st}_{c6}") for c6 in range(5)]
                vt = vps.tile([128, 4, 128], f32, tag="vt", name=f"vps{st}")
                for kq in range(7):
                    for c6 in range(5):
                        for kc in range(kq * 4, kq * 4 + 4):
                            nc.tensor.matmul(
                                qpss[c6][:],
                                lhsT=w_sb[:, kc, c6 * 128:(c6 + 1) * 128],
                                rhs=xb[:, kc, :],
                                start=(kc == 0), stop=False)
                    for kc in range(kq * 4, kq * 4 + 4):
                        for q in range(4):
                            # start only on the first matmul into the bank:
                            # start=True zeroes the whole 2KB region, which
                            # pre-zeroes all four q-group slices
                            nc.tensor.matmul(
                                vt[:, q, :],
                                lhsT=xb[:, kc, q * 128:(q + 1) * 128],
                                rhs=w_sb[:, kc, 640:768],
                                start=(kc == 0 and q == 0), stop=False)
                for c6 in range(5):
                    for kc in range(28, 32):
                        nc.tensor.matmul(
                            qpss[c6][:],
                            lhsT=w_sb[:, kc, c6 * 128:(c6 + 1) * 128],
                            rhs=xb[:, kc, :],
                            start=False, stop=(kc == KC - 1))
                    evict_c6(st, c6, qpss[c6], sl)
                for kc in range(28, 32):
                    for q in range(4):
                        nc.tensor.matmul(
                            vt[:, q, :],
                            lhsT=xb[:, kc, q * 128:(q + 1) * 128],
                            rhs=w_sb[:, kc, 640:768],
                            start=False, stop=(kc == KC - 1))
                nc.scalar.copy(out=v_sb[:, st * 4:(st + 1) * 4, :],
                               in_=vt[:])

            # ---- overlap window: st3 qkv as sequential per-column streams,
            # g0+g1+g2 attention pumped between chunks ----
            ps1_cm.__exit__(None, None, None)
            ops2_cm = tc.tile_pool(name="ps_o2", bufs=2, space="PSUM")
            ops2 = ops2_cm.__enter__()
            ps1b_cm = tc.tile_pool(name="ps_qkv3", bufs=3, space="PSUM")
            ps1b = ps1b_cm.__enter__()

            g12_stream = attn_stream([1, 0, 2], ops2, ps_s)
            st, xb, sl = 3, xbufs[3], slice(1536, 2048)
            vt = vps.tile([128, 4, 128], f32, tag="vt", name="vps3")
            for c6 in range(5):
                qps = ps1b.tile([128, 512], f32, tag="qkv3")
                for k4 in range(0, KC, 4):
                    for kc in range(k4, k4 + 4):
                        nc.tensor.matmul(
                            qps[:],
                            lhsT=w_sb[:, kc, c6 * 128:(c6 + 1) * 128],
                            rhs=xb[:, kc, :],
                            start=(kc == 0), stop=(kc == KC - 1))
                    pump(g12_stream, 2)
                evict_c6(st, c6, qps, sl)
            for k4 in range(0, KC, 4):
                for kc in range(k4, k4 + 4):
                    for q in range(4):
                        nc.tensor.matmul(
                            vt[:, q, :],
                            lhsT=xb[:, kc, q * 128:(q + 1) * 128],
                            rhs=w_sb[:, kc, 640:768],
                            start=(kc == 0 and q == 0),
                            stop=(kc == KC - 1))
                pump(g12_stream, 2)
            nc.scalar.copy(out=v_sb[:, 12:16, :], in_=vt[:])
            pump(g12_stream, 999)  # drain leftover steps

            # ---- free phase-1 pools; g3 attention + all y tiles ----
            ps1b_cm.__exit__(None, None, None)
            rp_cm.__exit__(None, None, None)
            ep_cm.__exit__(None, None, None)
            csn_cm.__exit__(None, None, None)
            xb_pool_cm.__exit__(None, None, None)
            w_pool_cm.__exit__(None, None, None)

            yp_cm = tc.tile_pool(name="yp", bufs=8)
            yp = yp_cm.__enter__()
            ps_y_cm = tc.tile_pool(name="ps_y", bufs=3, space="PSUM")
            ps_y = ps_y_cm.__enter__()

            ycnt = 0

            def y_tile(gy, ym):
                nonlocal ycnt
                gysl = slice(gy * 512, (gy + 1) * 512)
                yps = ps_y.tile([128, 512], f32, tag="yps")
                for kc in range(NQ):
                    nc.tensor.matmul(
                        yps[:],
                        lhsT=wo_sb[:, kc, ym * 128:(ym + 1) * 128],
                        rhs=outT_sb[:, kc, gysl],
                        start=(kc == 0), stop=(kc == NQ - 1))
                ysm = yp.tile([128, 512], f16, tag="ysm")
                if ycnt % 2 == 0:
                    nc.scalar.copy(out=ysm[:], in_=yps[:])
                else:
                    nc.vector.tensor_copy(out=ysm[:], in_=yps[:])
                nc.sync.dma_start(out=yT_v[:, ym:ym + 1, gysl], in_=ysm[:])
                ycnt += 1

            yq = [(g, ym) for g in range(3) for ym in range(32)]
            yi = 0

            def pump_y(n):
                nonlocal yi
                for _ in range(n):
                    if yi < len(yq):
                        y_tile(*yq[yi])
                        yi += 1

            g3_stream = attn_stream([3], ops2, ps_s)
            g3i = 0
            while next(g3_stream, "done") != "done":
                # slow the pump through the last head so a few non-g3 tiles
                # remain to bridge the final softmax-tail latency before the
                # y(3) tiles become available
                pump_y(1 if (g3i % 2 == 0 or g3i >= 48) else 2)
                g3i += 1
            yq.extend((3, ym) for ym in range(32))
            pump_y(len(yq))

            ps_y_cm.__exit__(None, None, None)
            yp_cm.__exit__(None, None, None)
            ops2_cm.__exit__(None, None, None)
            vps_cm.__exit__(None, None, None)

    nc.compile()
    return nc


def _get_nc():
    if "nc" not in _CACHE:
        _CACHE["nc"] = _build()
    return _CACHE["nc"]


def _prep_inputs(x, rope_cache, wqkv, wo):
    x2 = np.asarray(x, np.float32).reshape(S, H)
    xT = np.ascontiguousarray(x2.T.astype(np.float16))          # [H, S]

    rc = np.asarray(rope_cache, np.float32)                      # [S, 64, 2]
    c = rc[:, :, 0].T.astype(np.float16)                         # [64, S]
    s = rc[:, :, 1].T.astype(np.float16)
    cs2 = np.ascontiguousarray(np.vstack([c, c]))                # [128, S]
    sn2 = np.ascontiguousarray(np.vstack([-s, s]))

    # transposed causal mask (multiplicative): keep k <= q
    kk = np.arange(128)
    dmask = (kk[:, None] <= kk[None, :]).astype(np.float16)

    # rope pair permutation within each 128-wide head: evens then odds
    perm = np.concatenate([np.arange(0, 128, 2), np.arange(1, 128, 2)])

    wq = np.asarray(wqkv, np.float32)
    wo_f = np.asarray(wo, np.float32)

    in_maps = []
    for cid in range(N_CORES):
        cols = []
        for hh in range(NQ):
            qh = wq[:, cid * 512 + hh * 128: cid * 512 + (hh + 1) * 128]
            cols.append(qh[:, perm])
        kh = wq[:, H + cid * 128: H + (cid + 1) * 128]
        cols.append(kh[:, perm])
        vh = wq[:, H + 1024 + cid * 128: H + 1024 + (cid + 1) * 128]
        cols.append(vh)
        wcat = np.concatenate(cols, axis=1).astype(np.float16)
        in_maps.append({
            "xT": xT,
            "w": np.ascontiguousarray(wcat),
            "wo": np.ascontiguousarray(
                wo_f[cid * 512:(cid + 1) * 512, :].astype(np.float16)),
            "cs2": cs2,
            "sn2": sn2,
            "dmask": dmask,
        })
    return in_maps


def kernel(x, last_pos, mask, rope_cache, wqkv, wo):
    global LAST_RESULTS
    from concourse.bass_utils import run_bass_kernel_spmd

    nc = _get_nc()
    in_maps = _prep_inputs(x, rope_cache, wqkv, wo)

    res = run_bass_kernel_spmd(nc, in_maps, list(range(N_CORES)))
    LAST_RESULTS = res
    if res.exec_time_ns is not None:
        print(f"HW exec time: {res.exec_time_ns} ns")
    yT = res.results[0]["yT"].astype(np.float64)
    for c in range(1, N_CORES):
        yT = yT + res.results[c]["yT"]
    return np.ascontiguousarray(yT.T).reshape(1, S, H).astype(np.float32)


# revision 8
# speedup vs baseline: 1.0093x; 1.0026x over previous
"""Llama3 attention prefill kernel for 8 Trainium2 NeuronCores.

Sharding: tensor-parallel over heads. Core c owns Q heads 4c..4c+3 and KV
head c (GQA group), plus the matching wqkv columns / wo rows. Each core
computes a partial output y_c = attn_c @ wo_c; the host sums the partials.

Schedule (single TileContext, one long PE stream with software pipelining):
  0. PE warm-up dummies at t=0 ride out the 0.65->2.4 GHz DVFS ramp while
     the first w/x DMA chunks land.
  1. Supertiles st=0..2: qkvT = w^T x in transposed layout (q/k) plus v in
     natural [pos, d] layout via lhsT=x; eager per-column PSUM eviction +
     RoPE on DVE.  g=0's flash-attention steps interleave into st=2.
  2. Overlap window: st=3's qkv runs as sequential per-column streams
     (2 PSUM banks) while g=1 and g=2 attention steps pump between chunks.
  3. g=3 attention + all y^T = wo^T out^T tiles interleaved; the kernel
     tail is a single y tile evict+DMA.
  Attention: S^T = K_j^T Q_g per (k-block, 512-wide q group) -> exp gives
  P^T directly, causal diagonal via multiplicative mask, row sums on
  gpsimd, normalization fused into the outT eviction multiply.
"""

import os
import sys

for _p in ("/opt/trn_rl_repo", "/root/.axon_site/_ro/trn_rl_repo"):
    if os.path.isdir(_p) and _p not in sys.path:
        sys.path.insert(0, _p)

import numpy as np

S = 2048
H = 4096
HD = 128
NQ = 4            # q heads per core
MQKV = 768        # per-core qkv columns: 512 q + 128 k + 128 v
N_CORES = 8
KC = H // 128     # 32 contraction chunks for qkv
KT = S // 128     # 16 pos tiles
NG = S // 512     # 4 q groups of 512 positions
SCALE = 1.0 / float(np.sqrt(HD))

_CACHE = {}
LAST_RESULTS = None


def _build():
    import concourse.tile as tile
    from concourse import bacc, bass_isa, mybir

    f32 = mybir.dt.float32
    f16 = mybir.dt.float16
    Exp = mybir.ActivationFunctionType.Exp

    nc = bacc.Bacc("TRN2", target_bir_lowering=False, debug=False)

    xT_ap = nc.dram_tensor("xT", [H, S], f16, kind="ExternalInput").ap()
    w_ap = nc.dram_tensor("w", [H, MQKV], f16, kind="ExternalInput").ap()
    wo_ap = nc.dram_tensor("wo", [NQ * HD, H], f16, kind="ExternalInput").ap()
    cs_ap = nc.dram_tensor("cs2", [128, S], f16, kind="ExternalInput").ap()
    sn_ap = nc.dram_tensor("sn2", [128, S], f16, kind="ExternalInput").ap()
    dm_ap = nc.dram_tensor("dmask", [128, 128], f16, kind="ExternalInput").ap()
    yT_ap = nc.dram_tensor("yT", [H, S], f16, kind="ExternalOutput").ap()

    # DRAM views for batched DMA: [p, chunk, col]
    xT_v = xT_ap.rearrange("(a p) s -> p a s", p=128)      # [128, 32, 2048]
    w_v = w_ap.rearrange("(a p) c -> p a c", p=128)        # [128, 32, 768]
    wo_v = wo_ap.rearrange("(a p) c -> p a c", p=128)      # [128, 4, 4096]
    yT_v = yT_ap.rearrange("(a p) s -> p a s", p=128)      # [128, 32, 2048]

    with tile.TileContext(nc) as tc:
        from contextlib import ExitStack

        with ExitStack() as ctx:
            const = ctx.enter_context(tc.tile_pool(name="const", bufs=1))
            dmask = const.tile([128, 128], f16)
            # warm-up dummies read dmask before its DMA lands (WAR is
            # sequenced by the framework); contents are irrelevant.
            nc.gpsimd.memset(dmask[:], 0.0)

            # resident tensors
            res = ctx.enter_context(tc.tile_pool(name="res", bufs=1))
            qT_sb = res.tile([128, NQ, S], f16, name="qT_sb")
            kT_sb = res.tile([128, S], f16, name="kT_sb")
            v_sb = res.tile([128, KT, 128], f16, name="v_sb")
            outT_sb = res.tile([128, NQ, S], f16, name="outT_sb")

            # wo resident from the start; DMA'd during phase 1.
            # Right-side stack order: pools that die mid-program (w) sit on
            # top of the program-lifetime attention pools (pool frees are
            # LIFO per side).
            wo_pool = ctx.enter_context(
                tc.tile_pool(name="wo_pool", bufs=1, side="right"))
            wo_sb = wo_pool.tile([128, NQ, H], f16, name="wo_sb")

            # attention SBUF pools (live from st2 through the end)
            pp = ctx.enter_context(
                tc.tile_pool(name="pp", bufs=3, side="right"))
            rr = ctx.enter_context(
                tc.tile_pool(name="rr", bufs=2, side="right"))
            lp = ctx.enter_context(
                tc.tile_pool(name="lp", bufs=1, side="right"))

            w_pool_cm = tc.tile_pool(name="w_pool", bufs=1, side="right")
            w_pool = w_pool_cm.__enter__()
            w_sb = w_pool.tile([128, KC, MQKV], f16, name="w_sb")

            xb_pool_cm = tc.tile_pool(name="xb", bufs=2)
            xb_pool = xb_pool_cm.__enter__()

            # rope streaming pools (cs/sn stream per supertile: 2-deep)
            csn_cm = tc.tile_pool(name="csn", bufs=2)
            csn = csn_cm.__enter__()
            ep_cm = tc.tile_pool(name="ep", bufs=3)
            ep = ep_cm.__enter__()
            rp_cm = tc.tile_pool(name="rp", bufs=2)
            rp = rp_cm.__enter__()

            # PSUM: scores(2) + v(1) + qkv(5) = 8 during st0-2
            ps_s = ctx.enter_context(
                tc.tile_pool(name="ps_s", bufs=2, space="PSUM"))
            vps_cm = tc.tile_pool(name="ps_v", bufs=1, space="PSUM")
            vps = vps_cm.__enter__()
            ps1_cm = tc.tile_pool(name="ps_qkv", bufs=5, space="PSUM")
            ps1 = ps1_cm.__enter__()

            # PE warm-up: the tensor engine ramps 0.65->1.2->2.4 GHz over
            # ~3us of continuous execution.  Dummy matmuls that depend only
            # on the memset ride out the ramp while the first w/x DMA
            # chunks are in flight; results are never read.
            wps = vps.tile([128, 4, 128], f32, tag="vt", name="warm_ps")
            for wi in range(28):
                nc.tensor.matmul(wps[:, 0, :], lhsT=dmask[:], rhs=dmask[:],
                                 start=True, stop=True)

            # ---- DMA schedule (single shared engine; order = priority) ----
            xbufs = [xb_pool.tile([128, KC, 512], f16, tag="xb",
                                  name=f"xb{st}") for st in range(NG)]
            _edges = [0, 1, 2, 4, 6, 8, 12, 16, 20, 24, 28, 32]
            for _a, _b in zip(_edges[:-1], _edges[1:]):
                ksl = slice(_a, _b)
                nc.sync.dma_start(out=w_sb[:, ksl, :], in_=w_v[:, ksl, :])
                nc.sync.dma_start(out=xbufs[0][:, ksl, :],
                                  in_=xT_v[:, ksl, 0:512])
            for k8 in range(0, KC, 8):
                nc.sync.dma_start(out=xbufs[1][:, k8:k8 + 8, :],
                                  in_=xT_v[:, k8:k8 + 8, 512:1024])
            cs_ts, sn_ts = [], []
            for st in range(NG):
                cs_ts.append(csn.tile([128, 512], f16, tag="cs",
                                      name=f"cs{st}"))
                sn_ts.append(csn.tile([128, 512], f16, tag="sn",
                                      name=f"sn{st}"))
            stsl = [slice(st * 512, (st + 1) * 512) for st in range(NG)]
            # cs/sn for st0/st1 land before their rope; st2/st3 slices reuse
            # the buffers, so their DMAs (which wait on st0/st1 rope) are
            # queued last to keep the in-order DMA queue from head-blocking
            for st in (0, 1):
                nc.sync.dma_start(out=cs_ts[st][:], in_=cs_ap[:, stsl[st]])
                nc.sync.dma_start(out=sn_ts[st][:], in_=sn_ap[:, stsl[st]])
            nc.sync.dma_start(out=dmask[:], in_=dm_ap[:, :])
            nc.sync.dma_start(out=xbufs[2][:], in_=xT_v[:, :, 1024:1536])
            nc.sync.dma_start(out=xbufs[3][:], in_=xT_v[:, :, 1536:2048])
            nc.sync.dma_start(out=wo_sb[:], in_=wo_v[:, :, :])
            for st in (2, 3):
                nc.sync.dma_start(out=cs_ts[st][:], in_=cs_ap[:, stsl[st]])
                nc.sync.dma_start(out=sn_ts[st][:], in_=sn_ap[:, stsl[st]])

            # ---- helpers ----
            def evict_c6(st, c6, qps, sl):
                # rope: partner half via two half-partition copies
                E = ep.tile([128, 512], f16, tag="E")
                nc.scalar.copy(out=E[:], in_=qps[:])
                Esw = ep.tile([128, 512], f16, tag="Esw")
                nc.vector.tensor_copy(out=Esw[0:64, :], in_=E[64:128, :])
                nc.vector.tensor_copy(out=Esw[64:128, :], in_=E[0:64, :])
                t1 = rp.tile([128, 512], f16, tag="t1")
                nc.vector.tensor_mul(t1[:], E[:], cs_ts[st][:])
                t2 = rp.tile([128, 512], f16, tag="t2")
                nc.vector.tensor_mul(t2[:], Esw[:], sn_ts[st][:])
                dst = (qT_sb[:, c6, sl] if c6 < 4 else kT_sb[:, sl])
                nc.vector.tensor_add(dst, t1[:], t2[:])

            def attn_p(g, h, j, R, sp_pool):
                # P production: S^T matmul -> exp -> causal mask -> R add
                ing = (j // 4 == g)
                c0 = (j - 4 * g) * 128 if ing else 0
                sps = sp_pool.tile([128, 512], f32, tag="sps")
                nc.tensor.matmul(
                    sps[:, c0:],
                    lhsT=kT_sb[:, j * 128:(j + 1) * 128],
                    rhs=qT_sb[:, h, g * 512 + c0:(g + 1) * 512],
                    start=True, stop=True)
                P = pp.tile([128, 512], f16, tag="P")
                nc.scalar.activation(P[:, c0:], sps[:, c0:], Exp, scale=SCALE)
                if ing:
                    nc.vector.tensor_mul(
                        P[:, c0:c0 + 128], P[:, c0:c0 + 128], dmask[:])
                if j == 0:
                    nc.vector.tensor_copy(out=R[:], in_=P[:])
                else:
                    nc.vector.tensor_add(R[:, c0:], R[:, c0:], P[:, c0:])
                return P, c0

            def attn_pv(g, h, j, ops, P, c0):
                nc.tensor.matmul(
                    ops[:, c0:], lhsT=v_sb[:, j, :], rhs=P[:, c0:],
                    start=(j == 0), stop=(j == 4 * g + 3))

            def attn_tail(g, h, ops, R):
                # softmax denominators on Pool/DVE only; normalization is
                # fused into the outT eviction multiply
                gsl = slice(g * 512, (g + 1) * 512)
                l_bc = lp.tile([128, 512], f32, tag="lbc")
                nc.gpsimd.partition_all_reduce(
                    l_bc[:], R[:], channels=128,
                    reduce_op=bass_isa.ReduceOp.add)
                rbc = lp.tile([128, 512], f16, tag="rbc_sb")
                with nc.allow_low_precision(reason="1/l fits f16"):
                    nc.vector.reciprocal(rbc[:], l_bc[:])
                nc.vector.tensor_mul(outT_sb[:, h, gsl], ops[:], rbc[:])

            def attn_stream(groups, ops_pool, sp_pool):
                # generator, one yield per slot.  Each slot emits the PV of
                # the PREVIOUS j and the P-production of the current j, so
                # the PV always consumes a slot-old P (exp long finished)
                # and never blocks the PE wait queue.
                for g in groups:
                    for h in range(NQ):
                        ops = ops_pool.tile([128, 512], f32, tag="ops")
                        R = rr.tile([128, 512], f16, tag="R")
                        prev = None
                        for j in range(4 * g + 4):
                            if prev is not None:
                                attn_pv(g, h, j - 1, ops, *prev)
                            prev = attn_p(g, h, j, R, sp_pool)
                            yield
                        attn_pv(g, h, 4 * g + 3, ops, *prev)
                        attn_tail(g, h, ops, R)

            def pump(stream, n):
                for _ in range(n):
                    if next(stream, "done") == "done":
                        return

            # ---- supertiles 0..2: qkv + rope ----
            for st in range(3):
                xb = xbufs[st]
                sl = slice(st * 512, (st + 1) * 512)
                qpss = [ps1.tile([128, 512], f32, tag="qkv",
                                 name=f"qps{st}_{c6}") for c6 in range(5)]
                vt = vps.tile([128, 4, 128], f32, tag="vt", name=f"vps{st}")
                for kq in range(7):
                    for c6 in range(5):
                        for kc in range(kq * 4, kq * 4 + 4):
                            nc.tensor.matmul(
                                qpss[c6][:],
                                lhsT=w_sb[:, kc, c6 * 128:(c6 + 1) * 128],
                                rhs=xb[:, kc, :],
                                start=(kc == 0), stop=False)
                    for kc in range(kq * 4, kq * 4 + 4):
                        for q in range(4):
                            # start only on the first matmul into the bank:
                            # start=True zeroes the whole 2KB region, which
                            # pre-zeroes all four q-group slices
                            nc.tensor.matmul(
                                vt[:, q, :],
                                lhsT=xb[:, kc, q * 128:(q + 1) * 128],
                                rhs=w_sb[:, kc, 640:768],
                                start=(kc == 0 and q == 0), stop=False)
                for c6 in range(5):
                    for kc in range(28, 32):
                        nc.tensor.matmul(
                            qpss[c6][:],
                            lhsT=w_sb[:, kc, c6 * 128:(c6 + 1) * 128],
                            rhs=xb[:, kc, :],
                            start=False, stop=(kc == KC - 1))
                    evict_c6(st, c6, qpss[c6], sl)
                for kc in range(28, 32):
                    for q in range(4):
                        nc.tensor.matmul(
                            vt[:, q, :],
                            lhsT=xb[:, kc, q * 128:(q + 1) * 128],
                            rhs=w_sb[:, kc, 640:768],
                            start=False, stop=(kc == KC - 1))
                nc.scalar.copy(out=v_sb[:, st * 4:(st + 1) * 4, :],
                               in_=vt[:])

            # ---- overlap window: st3 qkv as sequential per-column streams,
            # g0+g1+g2 attention pumped between chunks ----
            ps1_cm.__exit__(None, None, None)
            ops2_cm = tc.tile_pool(name="ps_o2", bufs=2, space="PSUM")
            ops2 = ops2_cm.__enter__()
            ps1b_cm = tc.tile_pool(name="ps_qkv3", bufs=3, space="PSUM")
            ps1b = ps1b_cm.__enter__()

            g12_stream = attn_stream([1, 0, 2], ops2, ps_s)
            st, xb, sl = 3, xbufs[3], slice(1536, 2048)
            vt = vps.tile([128, 4, 128], f32, tag="vt", name="vps3")
            for c6 in range(5):
                qps = ps1b.tile([128, 512], f32, tag="qkv3")
                for k4 in range(0, KC, 4):
                    for kc in range(k4, k4 + 4):
                        nc.tensor.matmul(
                            qps[:],
                            lhsT=w_sb[:, kc, c6 * 128:(c6 + 1) * 128],
                            rhs=xb[:, kc, :],
                            start=(kc == 0), stop=(kc == KC - 1))
                    pump(g12_stream, 2)
                evict_c6(st, c6, qps, sl)
            for k4 in range(0, KC, 4):
                for kc in range(k4, k4 + 4):
                    for q in range(4):
                        nc.tensor.matmul(
                            vt[:, q, :],
                            lhsT=xb[:, kc, q * 128:(q + 1) * 128],
                            rhs=w_sb[:, kc, 640:768],
                            start=(kc == 0 and q == 0),
                            stop=(kc == KC - 1))
                pump(g12_stream, 2)
            nc.scalar.copy(out=v_sb[:, 12:16, :], in_=vt[:])
            pump(g12_stream, 999)  # drain leftover steps

            # ---- free phase-1 pools; g3 attention + all y tiles ----
            ps1b_cm.__exit__(None, None, None)
            rp_cm.__exit__(None, None, None)
            ep_cm.__exit__(None, None, None)
            csn_cm.__exit__(None, None, None)
            xb_pool_cm.__exit__(None, None, None)
            w_pool_cm.__exit__(None, None, None)

            yp_cm = tc.tile_pool(name="yp", bufs=8)
            yp = yp_cm.__enter__()
            ps_y_cm = tc.tile_pool(name="ps_y", bufs=3, space="PSUM")
            ps_y = ps_y_cm.__enter__()

            ycnt = 0

            def y_tile(gy, ym):
                nonlocal ycnt
                gysl = slice(gy * 512, (gy + 1) * 512)
                yps = ps_y.tile([128, 512], f32, tag="yps")
                for kc in range(NQ):
                    nc.tensor.matmul(
                        yps[:],
                        lhsT=wo_sb[:, kc, ym * 128:(ym + 1) * 128],
                        rhs=outT_sb[:, kc, gysl],
                        start=(kc == 0), stop=(kc == NQ - 1))
                ysm = yp.tile([128, 512], f16, tag="ysm")
                if ycnt % 2 == 0:
                    nc.scalar.copy(out=ysm[:], in_=yps[:])
                else:
                    nc.vector.tensor_copy(out=ysm[:], in_=yps[:])
                nc.sync.dma_start(out=yT_v[:, ym:ym + 1, gysl], in_=ysm[:])
                ycnt += 1

            yq = [(g, ym) for g in range(3) for ym in range(32)]
            yi = 0

            def pump_y(n):
                nonlocal yi
                for _ in range(n):
                    if yi < len(yq):
                        y_tile(*yq[yi])
                        yi += 1

            g3_stream = attn_stream([3], ops2, ps_s)
            g3i = 0
            while next(g3_stream, "done") != "done":
                # slow the pump through the last head so a few non-g3 tiles
                # remain to bridge the final softmax-tail latency before the
                # y(3) tiles become available
                pump_y((2 if g3i % 4 in (1, 2) else 1) if g3i < 46 else 1)
                g3i += 1
            yq.extend((3, ym) for ym in range(32))
            pump_y(len(yq))

            ps_y_cm.__exit__(None, None, None)
            yp_cm.__exit__(None, None, None)
            ops2_cm.__exit__(None, None, None)
            vps_cm.__exit__(None, None, None)

    nc.compile()
    return nc


def _get_nc():
    if "nc" not in _CACHE:
        _CACHE["nc"] = _build()
    return _CACHE["nc"]


def _prep_inputs(x, rope_cache, wqkv, wo):
    x2 = np.asarray(x, np.float32).reshape(S, H)
    xT = np.ascontiguousarray(x2.T.astype(np.float16))          # [H, S]

    rc = np.asarray(rope_cache, np.float32)                      # [S, 64, 2]
    c = rc[:, :, 0].T.astype(np.float16)                         # [64, S]
    s = rc[:, :, 1].T.astype(np.float16)
    cs2 = np.ascontiguousarray(np.vstack([c, c]))                # [128, S]
    sn2 = np.ascontiguousarray(np.vstack([-s, s]))

    # transposed causal mask (multiplicative): keep k <= q
    kk = np.arange(128)
    dmask = (kk[:, None] <= kk[None, :]).astype(np.float16)

    # rope pair permutation within each 128-wide head: evens then odds
    perm = np.concatenate([np.arange(0, 128, 2), np.arange(1, 128, 2)])

    wq = np.asarray(wqkv, np.float32)
    wo_f = np.asarray(wo, np.float32)

    in_maps = []
    for cid in range(N_CORES):
        cols = []
        for hh in range(NQ):
            qh = wq[:, cid * 512 + hh * 128: cid * 512 + (hh + 1) * 128]
            cols.append(qh[:, perm])
        kh = wq[:, H + cid * 128: H + (cid + 1) * 128]
        cols.append(kh[:, perm])
        vh = wq[:, H + 1024 + cid * 128: H + 1024 + (cid + 1) * 128]
        cols.append(vh)
        wcat = np.concatenate(cols, axis=1).astype(np.float16)
        in_maps.append({
            "xT": xT,
            "w": np.ascontiguousarray(wcat),
            "wo": np.ascontiguousarray(
                wo_f[cid * 512:(cid + 1) * 512, :].astype(np.float16)),
            "cs2": cs2,
            "sn2": sn2,
            "dmask": dmask,
        })
    return in_maps


def kernel(x, last_pos, mask, rope_cache, wqkv, wo):
    global LAST_RESULTS
    from concourse.bass_utils import run_bass_kernel_spmd

    nc = _get_nc()
    in_maps = _prep_inputs(x, rope_cache, wqkv, wo)

    res = run_bass_kernel_spmd(nc, in_maps, list(range(N_CORES)))
    LAST_RESULTS = res
    if res.exec_time_ns is not None:
        print(f"HW exec time: {res.exec_time_ns} ns")
    yT = res.results[0]["yT"].astype(np.float64)
    for c in range(1, N_CORES):
        yT = yT + res.results[c]["yT"]
    return np.ascontiguousarray(yT.T).reshape(1, S, H).astype(np.float32)


# revision 9
# speedup vs baseline: 1.0174x; 1.0080x over previous
"""Llama3 attention prefill kernel for 8 Trainium2 NeuronCores.

Sharding: tensor-parallel over heads. Core c owns Q heads 4c..4c+3 and KV
head c (GQA group), plus the matching wqkv columns / wo rows. Each core
computes a partial output y_c = attn_c @ wo_c; the host sums the partials.

Schedule (single TileContext, one long PE stream with software pipelining):
  0. PE warm-up dummies at t=0 ride out the 0.65->2.4 GHz DVFS ramp while
     the first w/x DMA chunks land.
  1. Supertiles st=0..2: qkvT = w^T x in transposed layout (q/k) plus v in
     natural [pos, d] layout via lhsT=x; eager per-column PSUM eviction +
     RoPE on DVE.  g=0's flash-attention steps interleave into st=2.
  2. Overlap window: st=3's qkv runs as sequential per-column streams
     (2 PSUM banks) while g=1 and g=2 attention steps pump between chunks.
  3. g=3 attention + all y^T = wo^T out^T tiles interleaved; the kernel
     tail is a single y tile evict+DMA.
  Attention: S^T = K_j^T Q_g per (k-block, 512-wide q group) -> exp gives
  P^T directly, causal diagonal via multiplicative mask, row sums on
  gpsimd, normalization fused into the outT eviction multiply.
"""

import os
import sys

for _p in ("/opt/trn_rl_repo", "/root/.axon_site/_ro/trn_rl_repo"):
    if os.path.isdir(_p) and _p not in sys.path:
        sys.path.insert(0, _p)

import numpy as np

S = 2048
H = 4096
HD = 128
NQ = 4            # q heads per core
MQKV = 768        # per-core qkv columns: 512 q + 128 k + 128 v
N_CORES = 8
KC = H // 128     # 32 contraction chunks for qkv
KT = S // 128     # 16 pos tiles
NG = S // 512     # 4 q groups of 512 positions
SCALE = 1.0 / float(np.sqrt(HD))

_CACHE = {}
LAST_RESULTS = None


def _build():
    import concourse.tile as tile
    from concourse import bacc, bass_isa, mybir

    f32 = mybir.dt.float32
    f16 = mybir.dt.float16
    Exp = mybir.ActivationFunctionType.Exp

    nc = bacc.Bacc("TRN2", target_bir_lowering=False, debug=False)

    xT_ap = nc.dram_tensor("xT", [H, S], f16, kind="ExternalInput").ap()
    w_ap = nc.dram_tensor("w", [H, MQKV], f16, kind="ExternalInput").ap()
    wo_ap = nc.dram_tensor("wo", [NQ * HD, H], f16, kind="ExternalInput").ap()
    cs_ap = nc.dram_tensor("cs2", [128, S], f16, kind="ExternalInput").ap()
    sn_ap = nc.dram_tensor("sn2", [128, S], f16, kind="ExternalInput").ap()
    dm_ap = nc.dram_tensor("dmask", [128, 128], f16, kind="ExternalInput").ap()
    yT_ap = nc.dram_tensor("yT", [H, S], f16, kind="ExternalOutput").ap()

    # DRAM views for batched DMA: [p, chunk, col]
    xT_v = xT_ap.rearrange("(a p) s -> p a s", p=128)      # [128, 32, 2048]
    w_v = w_ap.rearrange("(a p) c -> p a c", p=128)        # [128, 32, 768]
    wo_v = wo_ap.rearrange("(a p) c -> p a c", p=128)      # [128, 4, 4096]
    yT_v = yT_ap.rearrange("(a p) s -> p a s", p=128)      # [128, 32, 2048]

    with tile.TileContext(nc) as tc:
        from contextlib import ExitStack

        with ExitStack() as ctx:
            const = ctx.enter_context(tc.tile_pool(name="const", bufs=1))
            dmask = const.tile([128, 128], f16)
            # warm-up dummies read dmask before its DMA lands (WAR is
            # sequenced by the framework); contents are irrelevant.
            nc.gpsimd.memset(dmask[:], 0.0)

            # resident tensors
            res = ctx.enter_context(tc.tile_pool(name="res", bufs=1))
            qT_sb = res.tile([128, NQ, S], f16, name="qT_sb")
            kT_sb = res.tile([128, S], f16, name="kT_sb")
            v_sb = res.tile([128, KT, 128], f16, name="v_sb")
            outT_sb = res.tile([128, NQ, S], f16, name="outT_sb")

            # wo resident from the start; DMA'd during phase 1.
            # Right-side stack order: pools that die mid-program (w) sit on
            # top of the program-lifetime attention pools (pool frees are
            # LIFO per side).
            wo_pool = ctx.enter_context(
                tc.tile_pool(name="wo_pool", bufs=1, side="right"))
            wo_sb = wo_pool.tile([128, NQ, H], f16, name="wo_sb")

            # attention SBUF pools (live from st2 through the end)
            pp = ctx.enter_context(
                tc.tile_pool(name="pp", bufs=5, side="right"))
            rr = ctx.enter_context(
                tc.tile_pool(name="rr", bufs=1, side="right"))
            lp = ctx.enter_context(
                tc.tile_pool(name="lp", bufs=1, side="right"))

            w_pool_cm = tc.tile_pool(name="w_pool", bufs=1, side="right")
            w_pool = w_pool_cm.__enter__()
            w_sb = w_pool.tile([128, KC, MQKV], f16, name="w_sb")

            xb_pool_cm = tc.tile_pool(name="xb", bufs=2)
            xb_pool = xb_pool_cm.__enter__()

            # rope streaming pools (cs/sn stream per supertile: 2-deep)
            csn_cm = tc.tile_pool(name="csn", bufs=2)
            csn = csn_cm.__enter__()
            ep_cm = tc.tile_pool(name="ep", bufs=3)
            ep = ep_cm.__enter__()
            rp_cm = tc.tile_pool(name="rp", bufs=2)
            rp = rp_cm.__enter__()

            # PSUM: scores(2) + v(1) + qkv(5) = 8 during st0-2
            ps_s = ctx.enter_context(
                tc.tile_pool(name="ps_s", bufs=2, space="PSUM"))
            vps_cm = tc.tile_pool(name="ps_v", bufs=1, space="PSUM")
            vps = vps_cm.__enter__()
            ps1_cm = tc.tile_pool(name="ps_qkv", bufs=5, space="PSUM")
            ps1 = ps1_cm.__enter__()

            # PE warm-up: the tensor engine ramps 0.65->1.2->2.4 GHz over
            # ~3us of continuous execution.  Dummy matmuls that depend only
            # on the memset ride out the ramp while the first w/x DMA
            # chunks are in flight; results are never read.
            wps = vps.tile([128, 4, 128], f32, tag="vt", name="warm_ps")
            for wi in range(28):
                nc.tensor.matmul(wps[:, 0, :], lhsT=dmask[:], rhs=dmask[:],
                                 start=True, stop=True)

            # ---- DMA schedule (single shared engine; order = priority) ----
            xbufs = [xb_pool.tile([128, KC, 512], f16, tag="xb",
                                  name=f"xb{st}") for st in range(NG)]
            _edges = [0, 1, 2, 4, 6, 8, 12, 16, 20, 24, 28, 32]
            for _a, _b in zip(_edges[:-1], _edges[1:]):
                ksl = slice(_a, _b)
                nc.sync.dma_start(out=w_sb[:, ksl, :], in_=w_v[:, ksl, :])
                nc.sync.dma_start(out=xbufs[0][:, ksl, :],
                                  in_=xT_v[:, ksl, 0:512])
            for k8 in range(0, KC, 8):
                nc.sync.dma_start(out=xbufs[1][:, k8:k8 + 8, :],
                                  in_=xT_v[:, k8:k8 + 8, 512:1024])
            cs_ts, sn_ts = [], []
            for st in range(NG):
                cs_ts.append(csn.tile([128, 512], f16, tag="cs",
                                      name=f"cs{st}"))
                sn_ts.append(csn.tile([128, 512], f16, tag="sn",
                                      name=f"sn{st}"))
            stsl = [slice(st * 512, (st + 1) * 512) for st in range(NG)]
            # cs/sn for st0/st1 land before their rope; st2/st3 slices reuse
            # the buffers, so their DMAs (which wait on st0/st1 rope) are
            # queued last to keep the in-order DMA queue from head-blocking
            for st in (0, 1):
                nc.sync.dma_start(out=cs_ts[st][:], in_=cs_ap[:, stsl[st]])
                nc.sync.dma_start(out=sn_ts[st][:], in_=sn_ap[:, stsl[st]])
            nc.sync.dma_start(out=dmask[:], in_=dm_ap[:, :])
            nc.sync.dma_start(out=xbufs[2][:], in_=xT_v[:, :, 1024:1536])
            nc.sync.dma_start(out=xbufs[3][:], in_=xT_v[:, :, 1536:2048])
            nc.sync.dma_start(out=wo_sb[:], in_=wo_v[:, :, :])
            for st in (2, 3):
                nc.sync.dma_start(out=cs_ts[st][:], in_=cs_ap[:, stsl[st]])
                nc.sync.dma_start(out=sn_ts[st][:], in_=sn_ap[:, stsl[st]])

            # ---- helpers ----
            def evict_c6(st, c6, qps, sl):
                # rope: partner half via two half-partition copies
                E = ep.tile([128, 512], f16, tag="E")
                nc.scalar.copy(out=E[:], in_=qps[:])
                Esw = ep.tile([128, 512], f16, tag="Esw")
                nc.vector.tensor_copy(out=Esw[0:64, :], in_=E[64:128, :])
                nc.vector.tensor_copy(out=Esw[64:128, :], in_=E[0:64, :])
                t1 = rp.tile([128, 512], f16, tag="t1")
                nc.vector.tensor_mul(t1[:], E[:], cs_ts[st][:])
                t2 = rp.tile([128, 512], f16, tag="t2")
                nc.vector.tensor_mul(t2[:], Esw[:], sn_ts[st][:])
                dst = (qT_sb[:, c6, sl] if c6 < 4 else kT_sb[:, sl])
                nc.vector.tensor_add(dst, t1[:], t2[:])

            def attn_p(g, h, j, R, sp_pool):
                # P production: S^T matmul -> exp -> causal mask -> R add
                ing = (j // 4 == g)
                c0 = (j - 4 * g) * 128 if ing else 0
                sps = sp_pool.tile([128, 512], f32, tag="sps")
                nc.tensor.matmul(
                    sps[:, c0:],
                    lhsT=kT_sb[:, j * 128:(j + 1) * 128],
                    rhs=qT_sb[:, h, g * 512 + c0:(g + 1) * 512],
                    start=True, stop=True)
                P = pp.tile([128, 512], f16, tag="P")
                nc.scalar.activation(P[:, c0:], sps[:, c0:], Exp, scale=SCALE)
                if ing:
                    nc.vector.tensor_mul(
                        P[:, c0:c0 + 128], P[:, c0:c0 + 128], dmask[:])
                if j == 0:
                    nc.vector.tensor_copy(out=R[:], in_=P[:])
                else:
                    nc.vector.tensor_add(R[:, c0:], R[:, c0:], P[:, c0:])
                return P, c0

            def attn_pv(g, h, j, ops, P, c0):
                nc.tensor.matmul(
                    ops[:, c0:], lhsT=v_sb[:, j, :], rhs=P[:, c0:],
                    start=(j == 0), stop=(j == 4 * g + 3))

            def attn_tail(g, h, ops, R):
                # softmax denominators on Pool/DVE only; normalization is
                # fused into the outT eviction multiply
                gsl = slice(g * 512, (g + 1) * 512)
                l_bc = lp.tile([128, 512], f32, tag="lbc")
                nc.gpsimd.partition_all_reduce(
                    l_bc[:], R[:], channels=128,
                    reduce_op=bass_isa.ReduceOp.add)
                rbc = lp.tile([128, 512], f16, tag="rbc_sb")
                with nc.allow_low_precision(reason="1/l fits f16"):
                    nc.vector.reciprocal(rbc[:], l_bc[:])
                nc.vector.tensor_mul(outT_sb[:, h, gsl], ops[:], rbc[:])

            def attn_stream(groups, ops_pool, sp_pool):
                # generator, one yield per slot.  Each slot emits the PV of
                # the PREVIOUS j and the P-production of the current j, so
                # the PV always consumes a slot-old P (exp long finished)
                # and never blocks the PE wait queue.
                for g in groups:
                    for h in range(NQ):
                        ops = ops_pool.tile([128, 512], f32, tag="ops")
                        R = rr.tile([128, 512], f16, tag="R")
                        pend = []
                        for j in range(4 * g + 4):
                            for it in [p for p in pend if p[3] <= j]:
                                attn_pv(g, h, it[0], ops, it[1], it[2])
                                pend.remove(it)
                            P, c0 = attn_p(g, h, j, R, sp_pool)
                            ing = (j // 4 == g)
                            pend.append((j, P, c0, j + 4))
                            yield
                        for it in sorted(pend):
                            attn_pv(g, h, it[0], ops, it[1], it[2])
                        attn_tail(g, h, ops, R)

            def pump(stream, n):
                for _ in range(n):
                    if next(stream, "done") == "done":
                        return

            # ---- supertiles 0..2: qkv + rope ----
            for st in range(3):
                xb = xbufs[st]
                sl = slice(st * 512, (st + 1) * 512)
                qpss = [ps1.tile([128, 512], f32, tag="qkv",
                                 name=f"qps{st}_{c6}") for c6 in range(5)]
                vt = vps.tile([128, 4, 128], f32, tag="vt", name=f"vps{st}")
                for kq in range(7):
                    for c6 in range(5):
                        for kc in range(kq * 4, kq * 4 + 4):
                            nc.tensor.matmul(
                                qpss[c6][:],
                                lhsT=w_sb[:, kc, c6 * 128:(c6 + 1) * 128],
                                rhs=xb[:, kc, :],
                                start=(kc == 0), stop=False)
                    for kc in range(kq * 4, kq * 4 + 4):
                        for q in range(4):
                            # start only on the first matmul into the bank:
                            # start=True zeroes the whole 2KB region, which
                            # pre-zeroes all four q-group slices
                            nc.tensor.matmul(
                                vt[:, q, :],
                                lhsT=xb[:, kc, q * 128:(q + 1) * 128],
                                rhs=w_sb[:, kc, 640:768],
                                start=(kc == 0 and q == 0), stop=False)
                for c6 in range(5):
                    for kc in range(28, 32):
                        nc.tensor.matmul(
                            qpss[c6][:],
                            lhsT=w_sb[:, kc, c6 * 128:(c6 + 1) * 128],
                            rhs=xb[:, kc, :],
                            start=False, stop=(kc == KC - 1))
                    evict_c6(st, c6, qpss[c6], sl)
                for kc in range(28, 32):
                    for q in range(4):
                        nc.tensor.matmul(
                            vt[:, q, :],
                            lhsT=xb[:, kc, q * 128:(q + 1) * 128],
                            rhs=w_sb[:, kc, 640:768],
                            start=False, stop=(kc == KC - 1))
                nc.scalar.copy(out=v_sb[:, st * 4:(st + 1) * 4, :],
                               in_=vt[:])

            # ---- overlap window: st3 qkv as sequential per-column streams,
            # g0+g1+g2 attention pumped between chunks ----
            ps1_cm.__exit__(None, None, None)
            ops2_cm = tc.tile_pool(name="ps_o2", bufs=2, space="PSUM")
            ops2 = ops2_cm.__enter__()
            ps1b_cm = tc.tile_pool(name="ps_qkv3", bufs=3, space="PSUM")
            ps1b = ps1b_cm.__enter__()

            g12_stream = attn_stream([1, 0, 2], ops2, ps_s)
            st, xb, sl = 3, xbufs[3], slice(1536, 2048)
            vt = vps.tile([128, 4, 128], f32, tag="vt", name="vps3")
            for c6 in range(5):
                qps = ps1b.tile([128, 512], f32, tag="qkv3")
                for k4 in range(0, KC, 4):
                    for kc in range(k4, k4 + 4):
                        nc.tensor.matmul(
                            qps[:],
                            lhsT=w_sb[:, kc, c6 * 128:(c6 + 1) * 128],
                            rhs=xb[:, kc, :],
                            start=(kc == 0), stop=(kc == KC - 1))
                    pump(g12_stream, 2)
                evict_c6(st, c6, qps, sl)
            for k4 in range(0, KC, 4):
                for kc in range(k4, k4 + 4):
                    for q in range(4):
                        nc.tensor.matmul(
                            vt[:, q, :],
                            lhsT=xb[:, kc, q * 128:(q + 1) * 128],
                            rhs=w_sb[:, kc, 640:768],
                            start=(kc == 0 and q == 0),
                            stop=(kc == KC - 1))
                pump(g12_stream, 2)
            nc.scalar.copy(out=v_sb[:, 12:16, :], in_=vt[:])
            pump(g12_stream, 999)  # drain leftover steps

            # ---- free phase-1 pools; g3 attention + all y tiles ----
            ps1b_cm.__exit__(None, None, None)
            rp_cm.__exit__(None, None, None)
            ep_cm.__exit__(None, None, None)
            csn_cm.__exit__(None, None, None)
            xb_pool_cm.__exit__(None, None, None)
            w_pool_cm.__exit__(None, None, None)

            yp_cm = tc.tile_pool(name="yp", bufs=8)
            yp = yp_cm.__enter__()
            ps_y_cm = tc.tile_pool(name="ps_y", bufs=3, space="PSUM")
            ps_y = ps_y_cm.__enter__()

            ycnt = 0

            def y_tile(gy, ym):
                nonlocal ycnt
                gysl = slice(gy * 512, (gy + 1) * 512)
                yps = ps_y.tile([128, 512], f32, tag="yps")
                for kc in range(NQ):
                    nc.tensor.matmul(
                        yps[:],
                        lhsT=wo_sb[:, kc, ym * 128:(ym + 1) * 128],
                        rhs=outT_sb[:, kc, gysl],
                        start=(kc == 0), stop=(kc == NQ - 1))
                ysm = yp.tile([128, 512], f16, tag="ysm")
                if ycnt % 2 == 0:
                    nc.scalar.copy(out=ysm[:], in_=yps[:])
                else:
                    nc.vector.tensor_copy(out=ysm[:], in_=yps[:])
                nc.sync.dma_start(out=yT_v[:, ym:ym + 1, gysl], in_=ysm[:])
                ycnt += 1

            yq = [(g, ym) for g in range(3) for ym in range(32)]
            yi = 0

            def pump_y(n):
                nonlocal yi
                for _ in range(n):
                    if yi < len(yq):
                        y_tile(*yq[yi])
                        yi += 1

            g3_stream = attn_stream([3], ops2, ps_s)
            g3i = 0
            while next(g3_stream, "done") != "done":
                # slow the pump through the last head so a few non-g3 tiles
                # remain to bridge the final softmax-tail latency before the
                # y(3) tiles become available
                pump_y((2 if g3i % 4 in (1, 2) else 1) if g3i < 46 else 1)
                g3i += 1
            yq.extend((3, ym) for ym in range(32))
            pump_y(len(yq))

            ps_y_cm.__exit__(None, None, None)
            yp_cm.__exit__(None, None, None)
            ops2_cm.__exit__(None, None, None)
            vps_cm.__exit__(None, None, None)

    nc.compile()
    return nc


def _get_nc():
    if "nc" not in _CACHE:
        _CACHE["nc"] = _build()
    return _CACHE["nc"]


def _prep_inputs(x, rope_cache, wqkv, wo):
    x2 = np.asarray(x, np.float32).reshape(S, H)
    xT = np.ascontiguousarray(x2.T.astype(np.float16))          # [H, S]

    rc = np.asarray(rope_cache, np.float32)                      # [S, 64, 2]
    c = rc[:, :, 0].T.astype(np.float16)                         # [64, S]
    s = rc[:, :, 1].T.astype(np.float16)
    cs2 = np.ascontiguousarray(np.vstack([c, c]))                # [128, S]
    sn2 = np.ascontiguousarray(np.vstack([-s, s]))

    # transposed causal mask (multiplicative): keep k <= q
    kk = np.arange(128)
    dmask = (kk[:, None] <= kk[None, :]).astype(np.float16)

    # rope pair permutation within each 128-wide head: evens then odds
    perm = np.concatenate([np.arange(0, 128, 2), np.arange(1, 128, 2)])

    wq = np.asarray(wqkv, np.float32)
    wo_f = np.asarray(wo, np.float32)

    in_maps = []
    for cid in range(N_CORES):
        cols = []
        for hh in range(NQ):
            qh = wq[:, cid * 512 + hh * 128: cid * 512 + (hh + 1) * 128]
            cols.append(qh[:, perm])
        kh = wq[:, H + cid * 128: H + (cid + 1) * 128]
        cols.append(kh[:, perm])
        vh = wq[:, H + 1024 + cid * 128: H + 1024 + (cid + 1) * 128]
        cols.append(vh)
        wcat = np.concatenate(cols, axis=1).astype(np.float16)
        in_maps.append({
            "xT": xT,
            "w": np.ascontiguousarray(wcat),
            "wo": np.ascontiguousarray(
                wo_f[cid * 512:(cid + 1) * 512, :].astype(np.float16)),
            "cs2": cs2,
            "sn2": sn2,
            "dmask": dmask,
        })
    return in_maps


def kernel(x, last_pos, mask, rope_cache, wqkv, wo):
    global LAST_RESULTS
    from concourse.bass_utils import run_bass_kernel_spmd

    nc = _get_nc()
    in_maps = _prep_inputs(x, rope_cache, wqkv, wo)

    res = run_bass_kernel_spmd(nc, in_maps, list(range(N_CORES)))
    LAST_RESULTS = res
    if res.exec_time_ns is not None:
        print(f"HW exec time: {res.exec_time_ns} ns")
    yT = res.results[0]["yT"].astype(np.float64)
    for c in range(1, N_CORES):
        yT = yT + res.results[c]["yT"]
    return np.ascontiguousarray(yT.T).reshape(1, S, H).astype(np.float32)


# revision 10
# speedup vs baseline: 1.0198x; 1.0024x over previous
"""Llama3 attention prefill kernel for 8 Trainium2 NeuronCores.

Sharding: tensor-parallel over heads. Core c owns Q heads 4c..4c+3 and KV
head c (GQA group), plus the matching wqkv columns / wo rows. Each core
computes a partial output y_c = attn_c @ wo_c; the host sums the partials.

Schedule (single TileContext, one long PE stream with software pipelining):
  0. PE warm-up dummies at t=0 ride out the 0.65->2.4 GHz DVFS ramp while
     the first w/x DMA chunks land.
  1. Supertiles st=0..2: qkvT = w^T x in transposed layout (q/k) plus v in
     natural [pos, d] layout via lhsT=x; eager per-column PSUM eviction +
     RoPE on DVE.  g=0's flash-attention steps interleave into st=2.
  2. Overlap window: st=3's qkv runs as sequential per-column streams
     (2 PSUM banks) while g=1 and g=2 attention steps pump between chunks.
  3. g=3 attention + all y^T = wo^T out^T tiles interleaved; the kernel
     tail is a single y tile evict+DMA.
  Attention: S^T = K_j^T Q_g per (k-block, 512-wide q group) -> exp gives
  P^T directly, causal diagonal via multiplicative mask, row sums on
  gpsimd, normalization fused into the outT eviction multiply.
"""

import os
import sys

for _p in ("/opt/trn_rl_repo", "/root/.axon_site/_ro/trn_rl_repo"):
    if os.path.isdir(_p) and _p not in sys.path:
        sys.path.insert(0, _p)

import numpy as np

S = 2048
H = 4096
HD = 128
NQ = 4            # q heads per core
MQKV = 768        # per-core qkv columns: 512 q + 128 k + 128 v
N_CORES = 8
KC = H // 128     # 32 contraction chunks for qkv
KT = S // 128     # 16 pos tiles
NG = S // 512     # 4 q groups of 512 positions
SCALE = 1.0 / float(np.sqrt(HD))

_CACHE = {}
LAST_RESULTS = None


def _build():
    import concourse.tile as tile
    from concourse import bacc, bass_isa, mybir

    f32 = mybir.dt.float32
    f16 = mybir.dt.float16
    Exp = mybir.ActivationFunctionType.Exp

    nc = bacc.Bacc("TRN2", target_bir_lowering=False, debug=False)

    xT_ap = nc.dram_tensor("xT", [H, S], f16, kind="ExternalInput").ap()
    w_ap = nc.dram_tensor("w", [H, MQKV], f16, kind="ExternalInput").ap()
    wo_ap = nc.dram_tensor("wo", [NQ * HD, H], f16, kind="ExternalInput").ap()
    cs_ap = nc.dram_tensor("cs2", [128, S], f16, kind="ExternalInput").ap()
    sn_ap = nc.dram_tensor("sn2", [128, S], f16, kind="ExternalInput").ap()
    dm_ap = nc.dram_tensor("dmask", [128, 128], f16, kind="ExternalInput").ap()
    yT_ap = nc.dram_tensor("yT", [H, S], f16, kind="ExternalOutput").ap()

    # DRAM views for batched DMA: [p, chunk, col]
    xT_v = xT_ap.rearrange("(a p) s -> p a s", p=128)      # [128, 32, 2048]
    w_v = w_ap.rearrange("(a p) c -> p a c", p=128)        # [128, 32, 768]
    wo_v = wo_ap.rearrange("(a p) c -> p a c", p=128)      # [128, 4, 4096]
    yT_v = yT_ap.rearrange("(a p) s -> p a s", p=128)      # [128, 32, 2048]

    with tile.TileContext(nc) as tc:
        from contextlib import ExitStack

        with ExitStack() as ctx:
            const = ctx.enter_context(tc.tile_pool(name="const", bufs=1))
            dmask = const.tile([128, 128], f16)
            # warm-up dummies read dmask before its DMA lands (WAR is
            # sequenced by the framework); contents are irrelevant.
            nc.gpsimd.memset(dmask[:], 0.0)

            # resident tensors
            res = ctx.enter_context(tc.tile_pool(name="res", bufs=1))
            qT_sb = res.tile([128, NQ, S], f16, name="qT_sb")
            kT_sb = res.tile([128, S], f16, name="kT_sb")
            v_sb = res.tile([128, KT, 128], f16, name="v_sb")
            outT_sb = res.tile([128, NQ, S], f16, name="outT_sb")

            # wo resident from the start; DMA'd during phase 1.
            # Right-side stack order: pools that die mid-program (w) sit on
            # top of the program-lifetime attention pools (pool frees are
            # LIFO per side).
            wo_pool = ctx.enter_context(
                tc.tile_pool(name="wo_pool", bufs=1, side="right"))
            wo_sb = wo_pool.tile([128, NQ, H], f16, name="wo_sb")

            # attention SBUF pools (live from st2 through the end)
            pp = ctx.enter_context(
                tc.tile_pool(name="pp", bufs=6, side="right"))
            rr = ctx.enter_context(
                tc.tile_pool(name="rr", bufs=2, side="right"))
            lp = ctx.enter_context(
                tc.tile_pool(name="lp", bufs=1, side="right"))

            w_pool_cm = tc.tile_pool(name="w_pool", bufs=1, side="right")
            w_pool = w_pool_cm.__enter__()
            w_sb = w_pool.tile([128, KC, MQKV], f16, name="w_sb")

            xb_pool_cm = tc.tile_pool(name="xb", bufs=2)
            xb_pool = xb_pool_cm.__enter__()

            # rope streaming pools (cs/sn stream per supertile: 2-deep)
            csn_cm = tc.tile_pool(name="csn", bufs=2)
            csn = csn_cm.__enter__()
            ep_cm = tc.tile_pool(name="ep", bufs=3)
            ep = ep_cm.__enter__()
            rp_cm = tc.tile_pool(name="rp", bufs=1)
            rp = rp_cm.__enter__()

            # PSUM: scores(2) + v(1) + qkv(5) = 8 during st0-2
            ps_s = ctx.enter_context(
                tc.tile_pool(name="ps_s", bufs=2, space="PSUM"))
            vps_cm = tc.tile_pool(name="ps_v", bufs=1, space="PSUM")
            vps = vps_cm.__enter__()
            ps1_cm = tc.tile_pool(name="ps_qkv", bufs=5, space="PSUM")
            ps1 = ps1_cm.__enter__()

            # PE warm-up: the tensor engine ramps 0.65->1.2->2.4 GHz over
            # ~3us of continuous execution.  Dummy matmuls that depend only
            # on the memset ride out the ramp while the first w/x DMA
            # chunks are in flight; results are never read.
            wps = vps.tile([128, 4, 128], f32, tag="vt", name="warm_ps")
            for wi in range(28):
                nc.tensor.matmul(wps[:, 0, :], lhsT=dmask[:], rhs=dmask[:],
                                 start=True, stop=True)

            # ---- DMA schedule (single shared engine; order = priority) ----
            xbufs = [xb_pool.tile([128, KC, 512], f16, tag="xb",
                                  name=f"xb{st}") for st in range(NG)]
            _edges = [0, 1, 2, 4, 6, 8, 12, 16, 20, 24, 28, 32]
            for _a, _b in zip(_edges[:-1], _edges[1:]):
                ksl = slice(_a, _b)
                nc.sync.dma_start(out=w_sb[:, ksl, :], in_=w_v[:, ksl, :])
                nc.sync.dma_start(out=xbufs[0][:, ksl, :],
                                  in_=xT_v[:, ksl, 0:512])
            for k8 in range(0, KC, 8):
                nc.sync.dma_start(out=xbufs[1][:, k8:k8 + 8, :],
                                  in_=xT_v[:, k8:k8 + 8, 512:1024])
            cs_ts, sn_ts = [], []
            for st in range(NG):
                cs_ts.append(csn.tile([128, 512], f16, tag="cs",
                                      name=f"cs{st}"))
                sn_ts.append(csn.tile([128, 512], f16, tag="sn",
                                      name=f"sn{st}"))
            stsl = [slice(st * 512, (st + 1) * 512) for st in range(NG)]
            # cs/sn for st0/st1 land before their rope; st2/st3 slices reuse
            # the buffers, so their DMAs (which wait on st0/st1 rope) are
            # queued last to keep the in-order DMA queue from head-blocking
            for st in (0, 1):
                nc.sync.dma_start(out=cs_ts[st][:], in_=cs_ap[:, stsl[st]])
                nc.sync.dma_start(out=sn_ts[st][:], in_=sn_ap[:, stsl[st]])
            nc.sync.dma_start(out=dmask[:], in_=dm_ap[:, :])
            nc.sync.dma_start(out=xbufs[2][:], in_=xT_v[:, :, 1024:1536])
            nc.sync.dma_start(out=xbufs[3][:], in_=xT_v[:, :, 1536:2048])
            nc.sync.dma_start(out=wo_sb[:], in_=wo_v[:, :, :])
            for st in (2, 3):
                nc.sync.dma_start(out=cs_ts[st][:], in_=cs_ap[:, stsl[st]])
                nc.sync.dma_start(out=sn_ts[st][:], in_=sn_ap[:, stsl[st]])

            # ---- helpers ----
            def evict_c6(st, c6, qps, sl):
                # rope: partner half via two half-partition copies
                E = ep.tile([128, 512], f16, tag="E")
                nc.scalar.copy(out=E[:], in_=qps[:])
                Esw = ep.tile([128, 512], f16, tag="Esw")
                nc.vector.tensor_copy(out=Esw[0:64, :], in_=E[64:128, :])
                nc.vector.tensor_copy(out=Esw[64:128, :], in_=E[0:64, :])
                t1 = rp.tile([128, 512], f16, tag="t1")
                nc.vector.tensor_mul(t1[:], E[:], cs_ts[st][:])
                t2 = rp.tile([128, 512], f16, tag="t2")
                nc.vector.tensor_mul(t2[:], Esw[:], sn_ts[st][:])
                dst = (qT_sb[:, c6, sl] if c6 < 4 else kT_sb[:, sl])
                nc.vector.tensor_add(dst, t1[:], t2[:])

            def attn_p(g, h, j, R, sp_pool):
                # P production: S^T matmul -> exp -> causal mask -> R add
                ing = (j // 4 == g)
                c0 = (j - 4 * g) * 128 if ing else 0
                sps = sp_pool.tile([128, 512], f32, tag="sps")
                nc.tensor.matmul(
                    sps[:, c0:],
                    lhsT=kT_sb[:, j * 128:(j + 1) * 128],
                    rhs=qT_sb[:, h, g * 512 + c0:(g + 1) * 512],
                    start=True, stop=True)
                P = pp.tile([128, 512], f16, tag="P")
                nc.scalar.activation(P[:, c0:], sps[:, c0:], Exp, scale=SCALE)
                if ing:
                    nc.vector.tensor_mul(
                        P[:, c0:c0 + 128], P[:, c0:c0 + 128], dmask[:])
                if j == 0:
                    nc.vector.tensor_copy(out=R[:], in_=P[:])
                else:
                    nc.vector.tensor_add(R[:, c0:], R[:, c0:], P[:, c0:])
                return P, c0

            def attn_pv(g, h, j, ops, P, c0):
                nc.tensor.matmul(
                    ops[:, c0:], lhsT=v_sb[:, j, :], rhs=P[:, c0:],
                    start=(j == 0), stop=(j == 4 * g + 3))

            def attn_tail(g, h, ops, R):
                # softmax denominators on Pool/DVE only; normalization is
                # fused into the outT eviction multiply
                gsl = slice(g * 512, (g + 1) * 512)
                l_bc = lp.tile([128, 512], f32, tag="lbc")
                nc.gpsimd.partition_all_reduce(
                    l_bc[:], R[:], channels=128,
                    reduce_op=bass_isa.ReduceOp.add)
                rbc = lp.tile([128, 512], f16, tag="rbc_sb")
                with nc.allow_low_precision(reason="1/l fits f16"):
                    nc.vector.reciprocal(rbc[:], l_bc[:])
                nc.vector.tensor_mul(outT_sb[:, h, gsl], ops[:], rbc[:])

            def attn_stream(groups, ops_pool, sp_pool):
                # generator, one yield per slot.  Each slot emits the PV of
                # the PREVIOUS j and the P-production of the current j, so
                # the PV always consumes a slot-old P (exp long finished)
                # and never blocks the PE wait queue.
                for g in groups:
                    for h in range(NQ):
                        ops = ops_pool.tile([128, 512], f32, tag="ops")
                        R = rr.tile([128, 512], f16, tag="R")
                        pend = []
                        for j in range(4 * g + 4):
                            for it in [p for p in pend if p[3] <= j]:
                                attn_pv(g, h, it[0], ops, it[1], it[2])
                                pend.remove(it)
                            P, c0 = attn_p(g, h, j, R, sp_pool)
                            ing = (j // 4 == g)
                            pend.append((j, P, c0, j + 4))
                            yield
                        for it in sorted(pend):
                            attn_pv(g, h, it[0], ops, it[1], it[2])
                        attn_tail(g, h, ops, R)

            def pump(stream, n):
                for _ in range(n):
                    if next(stream, "done") == "done":
                        return

            # ---- supertiles 0..2: qkv + rope ----
            for st in range(3):
                xb = xbufs[st]
                sl = slice(st * 512, (st + 1) * 512)
                qpss = [ps1.tile([128, 512], f32, tag="qkv",
                                 name=f"qps{st}_{c6}") for c6 in range(5)]
                vt = vps.tile([128, 4, 128], f32, tag="vt", name=f"vps{st}")
                for kq in range(7):
                    for c6 in range(5):
                        for kc in range(kq * 4, kq * 4 + 4):
                            nc.tensor.matmul(
                                qpss[c6][:],
                                lhsT=w_sb[:, kc, c6 * 128:(c6 + 1) * 128],
                                rhs=xb[:, kc, :],
                                start=(kc == 0), stop=False)
                    for kc in range(kq * 4, kq * 4 + 4):
                        for q in range(4):
                            # start only on the first matmul into the bank:
                            # start=True zeroes the whole 2KB region, which
                            # pre-zeroes all four q-group slices
                            nc.tensor.matmul(
                                vt[:, q, :],
                                lhsT=xb[:, kc, q * 128:(q + 1) * 128],
                                rhs=w_sb[:, kc, 640:768],
                                start=(kc == 0 and q == 0), stop=False)
                for c6 in range(5):
                    for kc in range(28, 32):
                        nc.tensor.matmul(
                            qpss[c6][:],
                            lhsT=w_sb[:, kc, c6 * 128:(c6 + 1) * 128],
                            rhs=xb[:, kc, :],
                            start=False, stop=(kc == KC - 1))
                    evict_c6(st, c6, qpss[c6], sl)
                for kc in range(28, 32):
                    for q in range(4):
                        nc.tensor.matmul(
                            vt[:, q, :],
                            lhsT=xb[:, kc, q * 128:(q + 1) * 128],
                            rhs=w_sb[:, kc, 640:768],
                            start=False, stop=(kc == KC - 1))
                nc.scalar.copy(out=v_sb[:, st * 4:(st + 1) * 4, :],
                               in_=vt[:])

            # ---- overlap window: st3 qkv as sequential per-column streams,
            # g0+g1+g2 attention pumped between chunks ----
            ps1_cm.__exit__(None, None, None)
            ops2_cm = tc.tile_pool(name="ps_o2", bufs=2, space="PSUM")
            ops2 = ops2_cm.__enter__()
            ps1b_cm = tc.tile_pool(name="ps_qkv3", bufs=3, space="PSUM")
            ps1b = ps1b_cm.__enter__()

            g12_stream = attn_stream([1, 0, 2], ops2, ps_s)
            st, xb, sl = 3, xbufs[3], slice(1536, 2048)
            vt = vps.tile([128, 4, 128], f32, tag="vt", name="vps3")
            for c6 in range(5):
                qps = ps1b.tile([128, 512], f32, tag="qkv3")
                for k4 in range(0, KC, 4):
                    for kc in range(k4, k4 + 4):
                        nc.tensor.matmul(
                            qps[:],
                            lhsT=w_sb[:, kc, c6 * 128:(c6 + 1) * 128],
                            rhs=xb[:, kc, :],
                            start=(kc == 0), stop=(kc == KC - 1))
                    pump(g12_stream, 2)
                evict_c6(st, c6, qps, sl)
            for k4 in range(0, KC, 4):
                for kc in range(k4, k4 + 4):
                    for q in range(4):
                        nc.tensor.matmul(
                            vt[:, q, :],
                            lhsT=xb[:, kc, q * 128:(q + 1) * 128],
                            rhs=w_sb[:, kc, 640:768],
                            start=(kc == 0 and q == 0),
                            stop=(kc == KC - 1))
                pump(g12_stream, 2)
            nc.scalar.copy(out=v_sb[:, 12:16, :], in_=vt[:])
            pump(g12_stream, 999)  # drain leftover steps

            # ---- free phase-1 pools; g3 attention + all y tiles ----
            ps1b_cm.__exit__(None, None, None)
            rp_cm.__exit__(None, None, None)
            ep_cm.__exit__(None, None, None)
            csn_cm.__exit__(None, None, None)
            xb_pool_cm.__exit__(None, None, None)
            w_pool_cm.__exit__(None, None, None)

            yp_cm = tc.tile_pool(name="yp", bufs=8)
            yp = yp_cm.__enter__()
            ps_y_cm = tc.tile_pool(name="ps_y", bufs=3, space="PSUM")
            ps_y = ps_y_cm.__enter__()

            ycnt = 0

            def y_tile(gy, ym):
                nonlocal ycnt
                gysl = slice(gy * 512, (gy + 1) * 512)
                yps = ps_y.tile([128, 512], f32, tag="yps")
                for kc in range(NQ):
                    nc.tensor.matmul(
                        yps[:],
                        lhsT=wo_sb[:, kc, ym * 128:(ym + 1) * 128],
                        rhs=outT_sb[:, kc, gysl],
                        start=(kc == 0), stop=(kc == NQ - 1))
                ysm = yp.tile([128, 512], f16, tag="ysm")
                if ycnt % 2 == 0:
                    nc.scalar.copy(out=ysm[:], in_=yps[:])
                else:
                    nc.vector.tensor_copy(out=ysm[:], in_=yps[:])
                nc.sync.dma_start(out=yT_v[:, ym:ym + 1, gysl], in_=ysm[:])
                ycnt += 1

            yq = [(g, ym) for g in range(3) for ym in range(32)]
            yi = 0

            def pump_y(n):
                nonlocal yi
                for _ in range(n):
                    if yi < len(yq):
                        y_tile(*yq[yi])
                        yi += 1

            g3_stream = attn_stream([3], ops2, ps_s)
            g3i = 0
            while next(g3_stream, "done") != "done":
                # slow the pump through the last head so a few non-g3 tiles
                # remain to bridge the final softmax-tail latency before the
                # y(3) tiles become available
                pump_y((2 if g3i % 4 in (1, 2) else 1) if g3i < 46 else 1)
                g3i += 1
            yq.extend((3, ym) for ym in range(32))
            pump_y(len(yq))

            ps_y_cm.__exit__(None, None, None)
            yp_cm.__exit__(None, None, None)
            ops2_cm.__exit__(None, None, None)
            vps_cm.__exit__(None, None, None)

    nc.compile()
    return nc


def _get_nc():
    if "nc" not in _CACHE:
        _CACHE["nc"] = _build()
    return _CACHE["nc"]


def _prep_inputs(x, rope_cache, wqkv, wo):
    x2 = np.asarray(x, np.float32).reshape(S, H)
    xT = np.ascontiguousarray(x2.T.astype(np.float16))          # [H, S]

    rc = np.asarray(rope_cache, np.float32)                      # [S, 64, 2]
    c = rc[:, :, 0].T.astype(np.float16)                         # [64, S]
    s = rc[:, :, 1].T.astype(np.float16)
    cs2 = np.ascontiguousarray(np.vstack([c, c]))                # [128, S]
    sn2 = np.ascontiguousarray(np.vstack([-s, s]))

    # transposed causal mask (multiplicative): keep k <= q
    kk = np.arange(128)
    dmask = (kk[:, None] <= kk[None, :]).astype(np.float16)

    # rope pair permutation within each 128-wide head: evens then odds
    perm = np.concatenate([np.arange(0, 128, 2), np.arange(1, 128, 2)])

    wq = np.asarray(wqkv, np.float32)
    wo_f = np.asarray(wo, np.float32)

    in_maps = []
    for cid in range(N_CORES):
        cols = []
        for hh in range(NQ):
            qh = wq[:, cid * 512 + hh * 128: cid * 512 + (hh + 1) * 128]
            cols.append(qh[:, perm])
        kh = wq[:, H + cid * 128: H + (cid + 1) * 128]
        cols.append(kh[:, perm])
        vh = wq[:, H + 1024 + cid * 128: H + 1024 + (cid + 1) * 128]
        cols.append(vh)
        wcat = np.concatenate(cols, axis=1).astype(np.float16)
        in_maps.append({
            "xT": xT,
            "w": np.ascontiguousarray(wcat),
            "wo": np.ascontiguousarray(
                wo_f[cid * 512:(cid + 1) * 512, :].astype(np.float16)),
            "cs2": cs2,
            "sn2": sn2,
            "dmask": dmask,
        })
    return in_maps


def kernel(x, last_pos, mask, rope_cache, wqkv, wo):
    global LAST_RESULTS
    from concourse.bass_utils import run_bass_kernel_spmd

    nc = _get_nc()
    in_maps = _prep_inputs(x, rope_cache, wqkv, wo)

    res = run_bass_kernel_spmd(nc, in_maps, list(range(N_CORES)))
    LAST_RESULTS = res
    if res.exec_time_ns is not None:
        print(f"HW exec time: {res.exec_time_ns} ns")
    yT = res.results[0]["yT"].astype(np.float64)
    for c in range(1, N_CORES):
        yT = yT + res.results[c]["yT"]
    return np.ascontiguousarray(yT.T).reshape(1, S, H).astype(np.float32)
